# revision 1
# baseline (speedup 1.0000x reference)
"""GCN (2-layer GCNConv + global_add_pool + dense head) on 8 TRN2 cores.

Strategy (graph/data parallel, per sharding hint):
 - Nodes block-partitioned: core c owns rows [6250c, 6250(c+1)).
 - Edges partitioned by destination block; per-node slot lists padded to
   groups of 8 (dummy slots -> zero row of the feature table).
 - Per layer: own-block feature matmul h = x @ W, fold hs = h * dinv,
   AllGather hs -> full 50048-row fp32 table in every core's HBM.
 - Aggregation per 128-node dst window: chunked indirect gathers (128 slots
   per chunk) -> constant B8-variant matmuls produce 8-slot group sums in
   PSUM -> one-hot C matmuls (built on device via iota/is_equal) map groups
   to nodes; bias folded in as a K=1 matmul of sqrt(deg) x b.
 - Epilogue: x_next = relu((agg + hs_own) * dinv); layer-1 also transposes
   x_next per window (PE) for the layer-2 feature matmul; layer-2 feeds the
   pooling matmul (one-hot over graph ids) accumulated in PSUM.
 - Pooling partial sums are scattered to global graph rows via 4 one-hot
   matmuls, AllReduced, and the dense head + log_softmax runs redundantly
   on every core.
"""
import sys

sys.path.insert(0, "/opt/trn_rl_repo")

import math
import numpy as np

import concourse.bacc as bacc
import concourse.bass as bass
import concourse.mybir as mybir
import concourse.tile as tile
from concourse import bass2jax

P = 128
N_NODES = 50000
N_EDGES = 640000
DIM = 128
DIM_OUT = 64
NUM_GRAPHS = 512
NCORES = 8
NB = N_NODES // NCORES          # 6250 nodes per core
WPC = math.ceil(NB / P)         # 49 windows per core
NBP = WPC * P                   # 6272 padded
GS = 8                          # slots per group (legacy)
DG_EVERY = 4                    # route every DG_EVERY-th window via dma_gather
ZROW = N_NODES                  # zero row index in the table
TROWS = N_NODES + 48            # table rows (zero pad region)

fp32 = mybir.dt.float32
i32 = mybir.dt.int32


# ---------------------------------------------------------------- host prep
def preprocess(x, edge_index, x_batch):
    src = np.asarray(edge_index[0], dtype=np.int64)
    dst = np.asarray(edge_index[1], dtype=np.int64)
    xb = np.asarray(x_batch, dtype=np.int64)
    x = np.asarray(x, dtype=np.float32)

    edeg = np.bincount(dst, minlength=N_NODES)
    deg = 1.0 + edeg.astype(np.float32)
    dinv = (1.0 / np.sqrt(deg)).astype(np.float32)
    sqd = np.sqrt(deg).astype(np.float32)

    order = np.argsort(dst, kind="stable")
    src_sorted = src[order]
    starts = np.zeros(N_NODES + 1, np.int64)
    np.cumsum(edeg, out=starts[1:])

    # per-(core, window) edge counts -> common chunk grid (SPMD uniform)
    need = np.zeros((NCORES, WPC), np.int64)
    for c in range(NCORES):
        b = c * NB
        for w in range(WPC):
            lo, hi = b + w * P, b + min((w + 1) * P, NB)
            need[c, w] = max(1, math.ceil(int(edeg[lo:hi].sum()) / P))
    CW = need.max(axis=0)                              # chunks per window
    total_chunks = int(CW.sum())

    per_core = []
    for c in range(NCORES):
        b = c * NB
        idx_all = np.full((P, total_chunks), ZROW, np.int32)
        nodeof = np.full((P, total_chunks), -1.0, np.float32)
        ccol = 0
        for w in range(WPC):
            nreal = min((w + 1) * P, NB) - w * P
            lo = b + w * P
            nloc = np.repeat(np.arange(nreal), edeg[lo:lo + nreal])
            srcs = src_sorted[starts[lo]:starts[lo + nreal]]
            E_w = len(nloc)
            for j in range(CW[w]):
                s0, s1 = j * P, min((j + 1) * P, E_w)
                if s1 > s0:
                    idx_all[:s1 - s0, ccol + j] = srcs[s0:s1]
                    nodeof[:s1 - s0, ccol + j] = nloc[s0:s1]
            ccol += CW[w]

        # dg stream arrays: int16 pair indices (wrapped 16-partition layout)
        dg_windows = [w for w in range(WPC) if DG_EVERY and (w % DG_EVERY) == (DG_EVERY - 1)]
        dgc_total = sum(int(CW[w]) for w in dg_windows)
        idx16 = np.zeros((P, 8 * max(1, dgc_total)), np.int16)
        par = np.zeros((P, max(1, dgc_total)), np.float32)
        notpar = np.ones((P, max(1, dgc_total)), np.float32)
        ccums = np.concatenate([[0], np.cumsum(CW)]).astype(int)
        dgc = 0
        for w in dg_windows:
            cw = int(CW[w])
            srcs_w = idx_all[:, ccums[w]:ccums[w] + cw]          # [128, cw]
            for j in range(cw):
                flat = srcs_w[:, j].astype(np.int64)             # slot p
                i = (dgc + j) * P + np.arange(P)                 # global dg slot
                pr = (flat & 1).astype(np.float32)
                par[:, dgc + j] = pr
                notpar[:, dgc + j] = 1.0 - pr
                half = (flat // 2).astype(np.int16)
                # index i -> idxs[i%16, i//16]; store replicated on 128 parts
                ii = i % (P)                                      # = p
                wrap_r = (i % 16)
                wrap_c = (i // 16)
                idx16[wrap_r, wrap_c] = half
            dgc += cw
        if dgc_total:
            for rblk in range(1, 8):
                idx16[16 * rblk:16 * (rblk + 1), :8 * dgc_total] =                     idx16[:16, :8 * dgc_total]

        # per-window per-partition node params, padded windows -> 0 / 1e9
        nid = b + np.arange(NBP)
        ok = np.arange(NBP) < NB
        dinv_c = np.where(ok, dinv[np.minimum(nid, N_NODES - 1)], 0.0)
        sqd_c = np.where(ok, sqd[np.minimum(nid, N_NODES - 1)], 0.0)
        gmin = int(xb[b])
        xbs_c = np.where(ok, xb[np.minimum(nid, N_NODES - 1)] - gmin, 1e9)
        assert int(xb[b + NB - 1]) - gmin + 1 <= P
        # pooling scatter: block b4 one-hot scalar: gmin + lg - 128*b4
        pools = np.stack(
            [gmin + np.arange(P, dtype=np.float32) - P * b4 for b4 in range(4)],
            axis=1,
        ).astype(np.float32)                           # [128 local g, 4]

        xT = np.zeros((DIM, NBP), np.float32)
        xT[:, :NB] = x[b:b + NB].T

        per_core.append(dict(
            xT=xT,
            idx_all=idx_all,
            nodeof=nodeof,
            dinv2d=dinv_c.reshape(WPC, P).T.copy().astype(np.float32),
            sqd_row=sqd_c.reshape(1, NBP).astype(np.float32),
            xbshift=xbs_c.reshape(WPC, P).T.copy().astype(np.float32),
            pools=pools,
            idx16=idx16, par=par, notpar=notpar,
        ))

    # dg stream: windows routed to dma_gather (round-robin by DG_EVERY)
    dg_windows = [w for w in range(WPC) if DG_EVERY and (w % DG_EVERY) == (DG_EVERY - 1)]
    dg_cols = {}
    dgc = 0
    for w in dg_windows:
        dg_cols[w] = dgc
        dgc += int(CW[w])
    shared = dict(CW=CW, total_chunks=total_chunks,
                  dg_windows=dg_windows, dg_cols=dg_cols, dg_total=dgc)
    return per_core, shared


def const_inputs(W1, b1, W2, b2, Wh, bh):
    b8v = np.zeros((P, 8, P), np.float32)
    for jj in range(8):
        for g in range(16):
            b8v[GS * g:GS * (g + 1), jj, 16 * jj + g] = 1.0
    iota = np.tile(np.arange(P, dtype=np.float32)[None, :], (P, 1))
    iota16 = iota.astype(np.float16)
    ident = np.eye(P, dtype=np.float32)
    return dict(
        b8v=b8v, b8v16=b8v.astype(np.float16), iota=iota, iota16=iota16,
        ident=ident,
        W1=np.asarray(W1, np.float32), W2=np.asarray(W2, np.float32),
        Wh=np.asarray(Wh, np.float32),
        b1=np.asarray(b1, np.float32).reshape(1, DIM),
        b2=np.asarray(b2, np.float32).reshape(1, DIM),
        bh=np.asarray(bh, np.float32).reshape(1, DIM_OUT),
        ones512=np.ones((1, NUM_GRAPHS), np.float32),
    )


# ---------------------------------------------------------------- kernel
def build_kernel(shared, gather_mode="indirect", gather_bufs=32,
                 single_core=False, agg_fp16=True, repeat=1):
    CW = shared["CW"]
    total_chunks = shared["total_chunks"]

    nc = bacc.Bacc("TRN2", target_bir_lowering=False, debug=False,
                   enable_asserts=False,
                   num_devices=1 if single_core else NCORES)
    adt = mybir.dt.float16 if agg_fp16 else fp32

    # inputs
    d_xT = nc.dram_tensor("xT", [DIM, NBP], fp32, kind="ExternalInput")
    d_idx = nc.dram_tensor("idx_all", [P, total_chunks], i32, kind="ExternalInput")
    dg_windows = set(shared.get("dg_windows", []))
    dg_cols = shared.get("dg_cols", {})
    dg_total = int(shared.get("dg_total", 0))
    d_idx16 = nc.dram_tensor("idx16", [P, 8 * max(1, dg_total)],
                             mybir.dt.int16, kind="ExternalInput")
    d_par = nc.dram_tensor("par", [P, max(1, dg_total)], fp32,
                           kind="ExternalInput")
    d_npar = nc.dram_tensor("notpar", [P, max(1, dg_total)], fp32,
                            kind="ExternalInput")
    d_no = nc.dram_tensor("nodeof", [P, total_chunks], fp32, kind="ExternalInput")
    d_dinv = nc.dram_tensor("dinv2d", [P, WPC], fp32, kind="ExternalInput")
    d_sqd = nc.dram_tensor("sqd_row", [1, NBP], fp32, kind="ExternalInput")
    d_xbs = nc.dram_tensor("xbshift", [P, WPC], fp32, kind="ExternalInput")
    d_pools = nc.dram_tensor("pools", [P, 4], fp32, kind="ExternalInput")
    d_iota = nc.dram_tensor("iota", [P, P], fp32, kind="ExternalInput")
    d_iota16 = nc.dram_tensor("iota16", [P, P], mybir.dt.float16,
                              kind="ExternalInput")
    d_id = nc.dram_tensor("ident", [P, P], fp32, kind="ExternalInput")
    d_W = [nc.dram_tensor("W1", [DIM, DIM], fp32, kind="ExternalInput"),
           nc.dram_tensor("W2", [DIM, DIM], fp32, kind="ExternalInput")]
    d_b = [nc.dram_tensor("b1", [1, DIM], fp32, kind="ExternalInput"),
           nc.dram_tensor("b2", [1, DIM], fp32, kind="ExternalInput")]
    d_Wh = nc.dram_tensor("Wh", [DIM, DIM_OUT], fp32, kind="ExternalInput")
    d_bh = nc.dram_tensor("bh", [1, DIM_OUT], fp32, kind="ExternalInput")
    d_ones = nc.dram_tensor("ones512", [1, NUM_GRAPHS], fp32, kind="ExternalInput")

    d_out = nc.dram_tensor("out", [NUM_GRAPHS, DIM_OUT], fp32,
                           kind="ExternalOutput")

    # internal DRAM
    tbl = [nc.dram_tensor(f"table{l}", [TROWS, DIM], adt, addr_space="Shared")
           for l in range(2)]
    ag_in = [nc.dram_tensor(f"ag_in{l}", [NB, DIM], adt) for l in range(2)]
    ar_in = nc.dram_tensor("ar_in", [NUM_GRAPHS, DIM], fp32)
    ar_out = nc.dram_tensor("ar_out", [NUM_GRAPHS, DIM], fp32,
                            addr_space="Shared")

    with tile.TileContext(nc) as tc:
        with tc.tile_pool(name="const", bufs=1) as cst, \
             tc.tile_pool(name="big", bufs=1) as bigp, \
             tc.tile_pool(name="gath", bufs=gather_bufs) as gpool, \
             tc.tile_pool(name="dgp", bufs=2) as dgpool, \
             tc.tile_pool(name="cpool", bufs=12) as cpool, \
             tc.tile_pool(name="work", bufs=4) as wk, \
             tc.tile_pool(name="xtile", bufs=4) as xt, \
             tc.tile_pool(name="ps_feat", bufs=3, space="PSUM") as ps_feat, \
             tc.tile_pool(name="ps_out", bufs=4, space="PSUM") as ps_out, \
             tc.tile_pool(name="ps_aux", bufs=1, space="PSUM") as ps_aux:

            # ---- constants to SBUF
            idx_sb = bigp.tile([P, total_chunks], i32)
            nc.sync.dma_start(idx_sb[:], d_idx[:, :])
            idx16_sb = bigp.tile([P, 8 * max(1, dg_total)], mybir.dt.int16)
            nc.sync.dma_start(idx16_sb[:], d_idx16[:, :])
            par_sb = bigp.tile([P, max(1, dg_total)], fp32)
            nc.sync.dma_start(par_sb[:], d_par[:, :])
            npar_sb = bigp.tile([P, max(1, dg_total)], fp32)
            nc.sync.dma_start(npar_sb[:], d_npar[:, :])
            no_sb = bigp.tile([P, total_chunks], fp32)
            nc.sync.dma_start(no_sb[:], d_no[:, :])
            dinv_sb = cst.tile([P, WPC], fp32)
            nc.sync.dma_start(dinv_sb[:], d_dinv[:, :])
            sqd_sb = cst.tile([1, NBP], fp32)
            nc.sync.dma_start(sqd_sb[:], d_sqd[:, :])
            xbs_sb = cst.tile([P, WPC], fp32)
            nc.sync.dma_start(xbs_sb[:], d_xbs[:, :])
            pools_sb = cst.tile([P, 4], fp32)
            nc.sync.dma_start(pools_sb[:], d_pools[:, :])
            iota_sb = cst.tile([P, P], fp32)
            nc.sync.dma_start(iota_sb[:], d_iota[:, :])
            iota16_sb = cst.tile([P, P], mybir.dt.float16)
            nc.sync.dma_start(iota16_sb[:], d_iota16[:, :])
            id_sb = cst.tile([P, P], fp32)
            nc.sync.dma_start(id_sb[:], d_id[:, :])
            W_sb = []
            for l in range(2):
                t = cst.tile([DIM, DIM], fp32, name=f"W{l}_sb")
                nc.sync.dma_start(t[:], d_W[l][:, :])
                W_sb.append(t)
            b_sb = []
            for l in range(2):
                t = cst.tile([1, DIM], fp32, name=f"b{l}_sb")
                nc.sync.dma_start(t[:], d_b[l][:, :])
                b_sb.append(t)
            Wh_sb = cst.tile([DIM, DIM_OUT], fp32)
            nc.sync.dma_start(Wh_sb[:], d_Wh[:, :])
            bh_sb = cst.tile([1, DIM_OUT], fp32)
            nc.sync.dma_start(bh_sb[:], d_bh[:, :])
            ones_sb = cst.tile([1, NUM_GRAPHS], fp32)
            nc.sync.dma_start(ones_sb[:], d_ones[:, :])

            # zero pad rows of both tables
            zpad = cst.tile([48, DIM], adt)
            nc.vector.memset(zpad[:], 0.0)
            for l in range(2):
                nc.sync.dma_start(tbl[l][N_NODES:TROWS, :], zpad[:])

            # persistent SBUF
            hs_own = [bigp.tile([P, NBP], fp32, name=f"hs_own{l}")
                      for l in range(2)]
            xT2_sb = bigp.tile([P, NBP], fp32)     # layer-2 feature input

            gz = None
            if gather_mode == "none":
                gz = cst.tile([P, DIM], adt, name="gz")
                nc.vector.memset(gz[:], 0.25)

            # ================= repetitions (timing) =================
            for _rep in range(repeat):
              pool_ps = ps_aux.tile([P, DIM], fp32, space="PSUM", tag="aux",
                                    name=f"pool_ps_{_rep}")
              # ================= per layer =================
              for l in range(2):
                  # ---- feature matmul + hs + AG input
                  for w in range(WPC):
                      sl = slice(w * P, (w + 1) * P)
                      if l == 0:
                          xt_t = xt.tile([DIM, P], fp32, tag="xt")
                          nc.sync.dma_start(xt_t[:], d_xT[:, sl])
                          lhs = xt_t[:]
                      else:
                          lhs = xT2_sb[:, sl]
                      ph = ps_feat.tile([P, DIM], fp32, space="PSUM", tag="ph")
                      nc.tensor.matmul(out=ph[:], lhsT=lhs, rhs=W_sb[l][:],
                                       start=True, stop=True)
                      nc.vector.tensor_scalar(
                          out=hs_own[l][:, sl], in0=ph[:],
                          scalar1=dinv_sb[:, w:w + 1], scalar2=None,
                          op0=mybir.AluOpType.mult)
                      hi = min((w + 1) * P, NB)
                      if hi > w * P:
                          if agg_fp16:
                              hsc = wk.tile([P, DIM], adt, tag="hsc")
                              nc.vector.tensor_copy(hsc[:], hs_own[l][:, sl])
                              nc.sync.dma_start(ag_in[l][w * P:hi, :],
                                                hsc[:hi - w * P, :])
                          else:
                              nc.sync.dma_start(ag_in[l][w * P:hi, :],
                                                hs_own[l][:hi - w * P, sl])

                  # ---- AllGather the table
                  if single_core:
                      nc.sync.dma_start(tbl[l][0:NB, :], ag_in[l][:, :])
                  else:
                      nc.gpsimd.collective_compute(
                          "AllGather", mybir.AluOpType.bypass,
                          ins=[ag_in[l][:, :]],
                          outs=[tbl[l][0:N_NODES, :]],
                          replica_groups=[list(range(NCORES))])

                  # ---- aggregation windows: per-chunk one-hot matmuls
                  ccol = 0
                  for w in range(WPC):
                      cw = int(CW[w])
                      po = ps_out.tile([P, DIM], fp32, space="PSUM", tag="po")
                      nc.tensor.matmul(out=po[:],
                                       lhsT=sqd_sb[0:1, w * P:(w + 1) * P],
                                       rhs=b_sb[l][:], start=True, stop=False)
                      use_dg = (w in dg_windows) and agg_fp16 and \
                          gather_mode == "indirect" and not single_core
                      if use_dg:
                          dgc = dg_cols[w]
                          g2 = dgpool.tile([P, cw, 256], adt, tag="g2",
                                          name=f"g2_{l}_{w}")
                          nc.gpsimd.dma_gather(
                              out_ap=g2[:, :, :],
                              in_ap=tbl[l][:, :].rearrange(
                                  "(a b) c -> a (b c)", b=2),
                              idxs_ap=idx16_sb[:, 8 * dgc:8 * (dgc + cw)],
                              num_idxs=P * cw, num_idxs_reg=P * cw,
                              elem_size=256, single_packet=False)
                          for j in range(cw):
                              Ce = cpool.tile([P, P], adt, tag="C")
                              nc.vector.tensor_scalar(
                                  out=Ce[:], in0=iota16_sb[:],
                                  scalar1=no_sb[:, ccol + j:ccol + j + 1],
                                  scalar2=npar_sb[:, dgc + j:dgc + j + 1],
                                  op0=mybir.AluOpType.is_equal,
                                  op1=mybir.AluOpType.mult)
                              nc.tensor.matmul(out=po[:], lhsT=Ce[:],
                                               rhs=g2[:, j, 0:DIM],
                                               start=False, stop=False)
                              Co = cpool.tile([P, P], adt, tag="C")
                              nc.vector.tensor_scalar(
                                  out=Co[:], in0=iota16_sb[:],
                                  scalar1=no_sb[:, ccol + j:ccol + j + 1],
                                  scalar2=par_sb[:, dgc + j:dgc + j + 1],
                                  op0=mybir.AluOpType.is_equal,
                                  op1=mybir.AluOpType.mult)
                              nc.tensor.matmul(out=po[:], lhsT=Co[:],
                                               rhs=g2[:, j, DIM:256],
                                               start=False, stop=(j == cw - 1))
                      else:
                        for j in range(cw):
                          if gather_mode == "none":
                              gt_ = gz
                          else:
                              gt_ = gpool.tile([P, DIM], adt, tag="g")
                              if gather_mode == "indirect":
                                  nc.gpsimd.indirect_dma_start(
                                      out=gt_[:], out_offset=None,
                                      in_=tbl[l][:, :],
                                      in_offset=bass.IndirectOffsetOnAxis(
                                          ap=idx_sb[:, ccol + j:ccol + j + 1],
                                          axis=0))
                              else:
                                  row0 = ((ccol + j) * 131) % 49000
                                  eng = (nc.gpsimd if gather_mode == "swdge_contig"
                                         else nc.sync)
                                  eng.dma_start(gt_[:], tbl[l][row0:row0 + P, :])
                          C = cpool.tile([P, P], adt, tag="C")
                          nc.vector.tensor_scalar(
                              out=C[:], in0=iota16_sb[:],
                              scalar1=no_sb[:, ccol + j:ccol + j + 1],
                              scalar2=None, op0=mybir.AluOpType.is_equal)
                          nc.tensor.matmul(out=po[:], lhsT=C[:], rhs=gt_[:],
                                           start=False, stop=(j == cw - 1))
                      # epilogue
                      tsb = wk.tile([P, DIM], fp32, tag="tsb")
                      nc.vector.tensor_tensor(
                          out=tsb[:], in0=po[:],
                          in1=hs_own[l][:, w * P:(w + 1) * P],
                          op=mybir.AluOpType.add)
                      xn = wk.tile([P, DIM], fp32, tag="xn")
                      nc.scalar.activation(out=xn[:], in_=tsb[:],
                                           func=mybir.ActivationFunctionType.Relu,
                                           scale=dinv_sb[:, w:w + 1])
                      if l == 0:
                          ptr = ps_feat.tile([P, DIM], fp32, space="PSUM",
                                             tag="ph")
                          nc.tensor.transpose(out=ptr[:], in_=xn[:],
                                              identity=id_sb[:])
                          nc.vector.tensor_copy(xT2_sb[:, w * P:(w + 1) * P],
                                                ptr[:])
                      else:
                          Cg = wk.tile([P, P], fp32, tag="Cg")
                          nc.vector.tensor_scalar(
                              out=Cg[:], in0=iota_sb[:],
                              scalar1=xbs_sb[:, w:w + 1],
                              scalar2=None, op0=mybir.AluOpType.is_equal)
                          nc.tensor.matmul(out=pool_ps[:], lhsT=Cg[:],
                                           rhs=xn[:], start=(w == 0),
                                           stop=(w == WPC - 1))
                      ccol += cw

              # ================= pooling scatter + AllReduce =================
              pool_sb = wk.tile([P, DIM], fp32)
              nc.vector.tensor_copy(pool_sb[:], pool_ps[:])
              for b4 in range(4):
                  S = wk.tile([P, P], fp32, tag="S")
                  nc.vector.tensor_scalar(
                      out=S[:], in0=iota_sb[:],
                      scalar1=pools_sb[:, b4:b4 + 1],
                      scalar2=None, op0=mybir.AluOpType.is_equal)
                  pblk = ps_feat.tile([P, DIM], fp32, space="PSUM", tag="ph")
                  nc.tensor.matmul(out=pblk[:], lhsT=S[:], rhs=pool_sb[:],
                                   start=True, stop=True)
                  sblk = wk.tile([P, DIM], fp32, tag="sblk")
                  nc.vector.tensor_copy(sblk[:], pblk[:])
                  nc.sync.dma_start(ar_in[b4 * P:(b4 + 1) * P, :], sblk[:])
              if single_core:
                  nc.sync.dma_start(ar_out[:, :], ar_in[:, :])
              else:
                  nc.gpsimd.collective_compute(
                      "AllReduce", mybir.AluOpType.add,
                      ins=[ar_in[:, :]], outs=[ar_out[:, :]],
                      replica_groups=[list(range(NCORES))])

              # ================= head =================
              pooledT = bigp.tile([P, NUM_GRAPHS], fp32)
              for b4 in range(4):
                  pt = wk.tile([P, DIM], fp32, tag="pt")
                  nc.sync.dma_start(pt[:], ar_out[b4 * P:(b4 + 1) * P, :])
                  ptp = ps_feat.tile([P, DIM], fp32, space="PSUM", tag="ph")
                  nc.tensor.transpose(out=ptp[:], in_=pt[:], identity=id_sb[:])
                  nc.vector.tensor_copy(pooledT[:, b4 * P:(b4 + 1) * P], ptp[:])
              lgp = ps_aux.tile([DIM_OUT, NUM_GRAPHS], fp32, space="PSUM",
                                tag="aux")
              nc.tensor.matmul(out=lgp[:], lhsT=Wh_sb[:], rhs=pooledT[:],
                               start=True, stop=False)
              nc.tensor.matmul(out=lgp[:], lhsT=bh_sb[:], rhs=ones_sb[:],
                               start=False, stop=True)
              lgT = wk.tile([DIM_OUT, NUM_GRAPHS], fp32)
              nc.vector.tensor_copy(lgT[:], lgp[:])
              for b4 in range(4):
                  lp = ps_feat.tile([P, DIM_OUT], fp32, space="PSUM", tag="ph")
                  nc.tensor.transpose(out=lp[:],
                                      in_=lgT[:, b4 * P:(b4 + 1) * P],
                                      identity=id_sb[:DIM_OUT, :DIM_OUT])
                  lg = wk.tile([P, DIM_OUT], fp32, tag="lg")
                  nc.vector.tensor_copy(lg[:], lp[:])
                  # log_softmax over the 64 classes (free dim)
                  m = wk.tile([P, 1], fp32, tag="m")
                  nc.vector.tensor_reduce(out=m[:], in_=lg[:],
                                          op=mybir.AluOpType.max,
                                          axis=mybir.AxisListType.X)
                  mneg = wk.tile([P, 1], fp32, tag="mneg")
                  nc.vector.tensor_scalar(out=mneg[:], in0=m[:], scalar1=-1.0,
                                          scalar2=None,
                                          op0=mybir.AluOpType.mult)
                  e = wk.tile([P, DIM_OUT], fp32, tag="e")
                  se = wk.tile([P, 1], fp32, tag="se")
                  nc.scalar.activation(out=e[:], in_=lg[:],
                                       func=mybir.ActivationFunctionType.Exp,
                                       bias=mneg[:, :1], accum_out=se[:])
                  lse = wk.tile([P, 1], fp32, tag="lse")
                  nc.scalar.activation(out=lse[:], in_=se[:],
                                       func=mybir.ActivationFunctionType.Ln)
                  off = wk.tile([P, 1], fp32, tag="off")
                  nc.vector.tensor_tensor(out=off[:], in0=lse[:], in1=m[:],
                                          op=mybir.AluOpType.add)
                  o = wk.tile([P, DIM_OUT], fp32, tag="o")
                  nc.vector.tensor_scalar(out=o[:], in0=lg[:],
                                          scalar1=off[:, :1], scalar2=None,
                                          op0=mybir.AluOpType.subtract)
                  nc.sync.dma_start(d_out[b4 * P:(b4 + 1) * P, :], o[:])

    nc.compile()
    return nc



# ---------------------------------------------------------------- entry
def kernel(x, edge_index, x_batch, W1, b1, W2, b2, Wh, bh):
    """Full-input GCN kernel: shards nodes/edges across 8 NeuronCores."""
    from concourse.bass_utils import run_bass_kernel_spmd

    per_core, shared = preprocess(x, edge_index, x_batch)
    consts = const_inputs(W1, b1, W2, b2, Wh, bh)
    in_maps = [{**pc, **consts} for pc in per_core]
    nc = build_kernel(shared)
    declared = set()
    for alloc in nc.m.functions[0].allocations:
        if isinstance(alloc, mybir.MemoryLocationSet) and \
                alloc.kind == "ExternalInput":
            declared.add(alloc.memorylocations[0].name)
    in_maps = [{k: v for k, v in m.items() if k in declared} for m in in_maps]
    res = run_bass_kernel_spmd(nc, in_maps, core_ids=list(range(NCORES)))
    return np.asarray(res.results[0]["out"], dtype=np.float32)



# revision 6
# speedup vs baseline: 4.1981x; 4.1981x over previous
"""GCN (2-layer GCNConv + global_add_pool + dense head) on 8 TRN2 cores.

Strategy (graph/data parallel, per sharding hint):
 - Nodes block-partitioned: core c owns rows [6250c, 6250(c+1)).
 - Edges partitioned by destination block, sorted by (dst window, src half).
 - Per layer: h = x @ W in fp16, hs = h * dinv folded on the Act engine,
   AllGather hs -> full fp16 table in every core's HBM.
 - Aggregation: batched dma_gather (one 512B descriptor per edge) using an
   overlapping-stride table view (elem_step = 1 row, elem = 2 rows) so each
   slot holds its true source row; int16 index range handled by splitting
   the table into lo/hi half views.  Per 128-slot chunk: one one-hot matmul
   (C built on DVE via is_equal) accumulated into the window's PSUM.
 - Self-loop contribution via identity matmul of hs; bias via K=1 matmul of
   sqrt(deg) x b.  Epilogue relu(po * dinv) on Act; layer-1 epilogue also
   transposes x2 (PE) and immediately runs the layer-2 feature matmul.
 - Layer-2 windows accumulate global_add_pool via one-hot graph matmuls;
   pooled partials are scattered to graph rows, AllReduced, and the dense
   head + log_softmax runs redundantly on every core.
"""
import sys

sys.path.insert(0, "/opt/trn_rl_repo")

import math
import numpy as np

import concourse.bacc as bacc
import concourse.bass as bass
import concourse.mybir as mybir
import concourse.tile as tile

P = 128
N_NODES = 50000
N_EDGES = 640000
DIM = 128
DIM_OUT = 64
NUM_GRAPHS = 512
NCORES = 8
NB = N_NODES // NCORES          # 6250 nodes per core
WPC = math.ceil(NB / P)         # 49 windows per core
NBP = WPC * P                   # 6272 padded
HALF = 25024                    # lo rows [0, HALF), hi rows [HALF, 50048)
TROWS = N_NODES + 56            # table rows incl. overfetch pad
WGRP = 4                        # windows per gather batch

fp32 = mybir.dt.float32
fp16 = mybir.dt.float16
i16 = mybir.dt.int16


# ---------------------------------------------------------------- host prep
def preprocess(x, edge_index, x_batch):
    src = np.asarray(edge_index[0], dtype=np.int64)
    dst = np.asarray(edge_index[1], dtype=np.int64)
    xb = np.asarray(x_batch, dtype=np.int64)
    x = np.asarray(x, dtype=np.float32)

    edeg = np.bincount(dst, minlength=N_NODES)
    deg = 1.0 + edeg.astype(np.float32)
    dinv = (1.0 / np.sqrt(deg)).astype(np.float32)
    sqd = np.sqrt(deg).astype(np.float32)

    order = np.argsort(dst, kind="stable")
    src_sorted = src[order]
    starts = np.zeros(N_NODES + 1, np.int64)
    np.cumsum(edeg, out=starts[1:])

    # per (core, window, half) edge lists; chunk grid = max over cores
    lists = [[None] * WPC for _ in range(NCORES)]
    cwlo = np.zeros((NCORES, WPC), np.int64)
    cwhi = np.zeros((NCORES, WPC), np.int64)
    for c in range(NCORES):
        b = c * NB
        for w in range(WPC):
            lo, hi = b + w * P, b + min((w + 1) * P, NB)
            srcs_w = src_sorted[starts[lo]:starts[hi]]
            nloc_w = np.repeat(np.arange(hi - lo), edeg[lo:hi])
            m = srcs_w < HALF
            lists[c][w] = (srcs_w[m], nloc_w[m], srcs_w[~m] - HALF, nloc_w[~m])
            cwlo[c, w] = max(1, math.ceil(int(m.sum()) / P))
            cwhi[c, w] = max(1, math.ceil(int((~m).sum()) / P))
    CWlo = cwlo.max(axis=0)
    CWhi = cwhi.max(axis=0)
    TClo = int(CWlo.sum())
    TChi = int(CWhi.sum())

    def wrap16(flat):
        # index i -> [i % 16, i // 16], replicated across 128 partitions
        n = len(flat)
        arr = np.zeros((P, n // 16), np.int16)
        arr[:16] = flat.reshape(n // 16, 16).T
        for r in range(1, 8):
            arr[16 * r:16 * (r + 1)] = arr[:16]
        return arr

    per_core = []
    for c in range(NCORES):
        b = c * NB
        streams = {}
        for half, CW, TC, ilo in (("lo", CWlo, TClo, 0), ("hi", CWhi, TChi, 2)):
            idxf = np.zeros(TC * P, np.int16)
            nof = np.full(TC * P, -1.0, np.float32)
            col = 0
            for w in range(WPC):
                s, n = lists[c][w][ilo], lists[c][w][ilo + 1]
                o = col * P
                idxf[o:o + len(s)] = s.astype(np.int16)
                nof[o:o + len(n)] = n.astype(np.float16)
                col += int(CW[w])
            streams[f"idx_{half}"] = wrap16(idxf)
            streams[f"no_{half}"] = nof.reshape(TC, P).T.copy()

        nid = b + np.arange(NBP)
        ok = np.arange(NBP) < NB
        nidc = np.minimum(nid, N_NODES - 1)
        dinv_c = np.where(ok, dinv[nidc], 0.0).astype(np.float32)
        sqd_c = np.where(ok, sqd[nidc], 0.0).astype(np.float16)
        gmin = int(xb[b])
        xbs_c = np.where(ok, xb[nidc] - gmin, 200.0).astype(np.float32)
        assert int(xb[b + NB - 1]) - gmin + 1 <= P
        pools = np.stack(
            [gmin + np.arange(P, dtype=np.float32) - P * b4 for b4 in range(4)],
            axis=1,
        ).astype(np.float32)

        xT = np.zeros((DIM, NBP), np.float16)
        xT[:, :NB] = x[b:b + NB].T.astype(np.float16)

        per_core.append(dict(
            xT=xT,
            dinv2d=dinv_c.reshape(WPC, P).T.copy(),
            sqdT=sqd_c.reshape(1, NBP),
            xbshift=xbs_c.reshape(WPC, P).T.copy(),
            pools=pools,
            **streams,
        ))

    shared = dict(CWlo=CWlo, CWhi=CWhi, TClo=TClo, TChi=TChi)
    return per_core, shared


def const_inputs(W1, b1, W2, b2, Wh, bh):
    iota = np.tile(np.arange(P, dtype=np.float32)[None, :], (P, 1))
    return dict(
        iota=iota, iota16=iota.astype(np.float16),
        ident16=np.eye(P, dtype=np.float16),
        ident=np.eye(P, dtype=np.float32),
        W1=np.asarray(W1, np.float16), W2=np.asarray(W2, np.float16),
        Wh=np.asarray(Wh, np.float32),
        b1=np.asarray(b1, np.float16).reshape(1, DIM),
        b2=np.asarray(b2, np.float16).reshape(1, DIM),
        bh=np.asarray(bh, np.float32).reshape(1, DIM_OUT),
        ones512=np.ones((1, NUM_GRAPHS), np.float32),
    )


# ---------------------------------------------------------------- kernel
def build_kernel(shared, single_core=False, wgrp=WGRP):
    CWlo, CWhi = shared["CWlo"], shared["CWhi"]
    TClo, TChi = shared["TClo"], shared["TChi"]

    nc = bacc.Bacc("TRN2", target_bir_lowering=False, debug=False,
                   enable_asserts=False,
                   num_devices=1 if single_core else NCORES)

    # inputs
    d_xT = nc.dram_tensor("xT", [DIM, NBP], fp16, kind="ExternalInput")
    d_idx = {h: nc.dram_tensor(f"idx_{h}", [P, tc * 8], i16,
                               kind="ExternalInput")
             for h, tc in (("lo", TClo), ("hi", TChi))}
    d_no = {h: nc.dram_tensor(f"no_{h}", [P, tc], fp32, kind="ExternalInput")
            for h, tc in (("lo", TClo), ("hi", TChi))}
    d_dinv = nc.dram_tensor("dinv2d", [P, WPC], fp32, kind="ExternalInput")
    d_sqd = nc.dram_tensor("sqdT", [1, NBP], fp16, kind="ExternalInput")
    d_xbs = nc.dram_tensor("xbshift", [P, WPC], fp32, kind="ExternalInput")
    d_pools = nc.dram_tensor("pools", [P, 4], fp32, kind="ExternalInput")
    d_iota = nc.dram_tensor("iota", [P, P], fp32, kind="ExternalInput")
    d_iota16 = nc.dram_tensor("iota16", [P, P], fp16, kind="ExternalInput")
    d_id16 = nc.dram_tensor("ident16", [P, P], fp16, kind="ExternalInput")
    d_id = nc.dram_tensor("ident", [P, P], fp32, kind="ExternalInput")
    d_W = [nc.dram_tensor("W1", [DIM, DIM], fp16, kind="ExternalInput"),
           nc.dram_tensor("W2", [DIM, DIM], fp16, kind="ExternalInput")]
    d_b = [nc.dram_tensor("b1", [1, DIM], fp16, kind="ExternalInput"),
           nc.dram_tensor("b2", [1, DIM], fp16, kind="ExternalInput")]
    d_Wh = nc.dram_tensor("Wh", [DIM, DIM_OUT], fp32, kind="ExternalInput")
    d_bh = nc.dram_tensor("bh", [1, DIM_OUT], fp32, kind="ExternalInput")
    d_ones = nc.dram_tensor("ones512", [1, NUM_GRAPHS], fp32,
                            kind="ExternalInput")

    d_out = nc.dram_tensor("out", [NUM_GRAPHS, DIM_OUT], fp32,
                           kind="ExternalOutput")

    # internal DRAM
    tbl = [nc.dram_tensor(f"table{l}", [TROWS, DIM], fp16, addr_space="Shared")
           for l in range(2)]
    ag_in = [nc.dram_tensor(f"ag_in{l}", [NB, DIM], fp16) for l in range(2)]
    ar_in = nc.dram_tensor("ar_in", [NUM_GRAPHS, DIM], fp32)
    ar_out = nc.dram_tensor("ar_out", [NUM_GRAPHS, DIM], fp32,
                            addr_space="Shared")

    # gather batches: [(w0, nw, col0_lo, cols_lo, col0_hi, cols_hi)]
    batches = []
    clo = chi = 0
    for w0 in range(0, WPC, wgrp):
        nw = min(wgrp, WPC - w0)
        cl = int(CWlo[w0:w0 + nw].sum())
        ch = int(CWhi[w0:w0 + nw].sum())
        batches.append((w0, nw, clo, cl, chi, ch))
        clo += cl
        chi += ch

    with tile.TileContext(nc) as tc:
        with tc.tile_pool(name="const", bufs=1) as cst, \
             tc.tile_pool(name="big", bufs=1) as bigp, \
             tc.tile_pool(name="glo", bufs=2) as glo_pool, \
             tc.tile_pool(name="ghi", bufs=2) as ghi_pool, \
             tc.tile_pool(name="cpool", bufs=12) as cpool, \
             tc.tile_pool(name="work", bufs=4) as wk, \
             tc.tile_pool(name="ps_feat", bufs=3, space="PSUM") as ps_feat, \
             tc.tile_pool(name="ps_out", bufs=4, space="PSUM") as ps_out, \
             tc.tile_pool(name="ps_aux", bufs=1, space="PSUM") as ps_aux:

            # ---- constants / inputs to SBUF
            xT_sb = bigp.tile([DIM, NBP], fp16)
            nc.sync.dma_start(xT_sb[:], d_xT[:, :])
            idx_sb = {}
            no_sb = {}
            for h, tc_ in (("lo", TClo), ("hi", TChi)):
                t = bigp.tile([P, tc_ * 8], i16, name=f"idx{h}_sb")
                nc.sync.dma_start(t[:], d_idx[h][:, :])
                idx_sb[h] = t
                t = bigp.tile([P, tc_], fp32, name=f"no{h}_sb")
                nc.sync.dma_start(t[:], d_no[h][:, :])
                no_sb[h] = t
            dinv_sb = cst.tile([P, WPC], fp32)
            nc.sync.dma_start(dinv_sb[:], d_dinv[:, :])
            sqd_sb = cst.tile([1, NBP], fp16)
            nc.sync.dma_start(sqd_sb[:], d_sqd[:, :])
            xbs_sb = cst.tile([P, WPC], fp32)
            nc.sync.dma_start(xbs_sb[:], d_xbs[:, :])
            pools_sb = cst.tile([P, 4], fp32)
            nc.sync.dma_start(pools_sb[:], d_pools[:, :])
            iota_sb = cst.tile([P, P], fp32)
            nc.sync.dma_start(iota_sb[:], d_iota[:, :])
            iota16_sb = cst.tile([P, P], fp16)
            nc.sync.dma_start(iota16_sb[:], d_iota16[:, :])
            id16_sb = cst.tile([P, P], fp16)
            nc.sync.dma_start(id16_sb[:], d_id16[:, :])
            id_sb = cst.tile([P, P], fp32)
            nc.sync.dma_start(id_sb[:], d_id[:, :])
            W_sb = []
            for l in range(2):
                t = cst.tile([DIM, DIM], fp16, name=f"W{l}_sb")
                nc.sync.dma_start(t[:], d_W[l][:, :])
                W_sb.append(t)
            b_sb = []
            for l in range(2):
                t = cst.tile([1, DIM], fp16, name=f"b{l}_sb")
                nc.sync.dma_start(t[:], d_b[l][:, :])
                b_sb.append(t)
            Wh_sb = cst.tile([DIM, DIM_OUT], fp32)
            nc.sync.dma_start(Wh_sb[:], d_Wh[:, :])
            bh_sb = cst.tile([1, DIM_OUT], fp32)
            nc.sync.dma_start(bh_sb[:], d_bh[:, :])
            ones_sb = cst.tile([1, NUM_GRAPHS], fp32)
            nc.sync.dma_start(ones_sb[:], d_ones[:, :])

            hs_sb = [bigp.tile([P, NBP], fp16, name=f"hs{l}") for l in range(2)]

            # overlapped-stride gather views: elem = 2 rows, step = 1 row
            gview = {}
            for h, base in (("lo", 0), ("hi", HALF)):
                gview[h] = [
                    bass.AP(tbl[l][base:, :].tensor, tbl[l][base:, :].offset,
                            [[DIM, HALF + 1], [1, 2 * DIM]])
                    for l in range(2)
                ]

            # ---- layer-1 features: hs1 = (x @ W1) * dinv
            for w in range(WPC):
                sl = slice(w * P, (w + 1) * P)
                ph = ps_feat.tile([P, DIM], fp32, space="PSUM", tag="ph")
                nc.tensor.matmul(out=ph[:], lhsT=xT_sb[:, sl], rhs=W_sb[0][:],
                                 start=True, stop=True)
                nc.scalar.activation(out=hs_sb[0][:, sl], in_=ph[:],
                                     func=mybir.ActivationFunctionType.Copy,
                                     scale=dinv_sb[:, w:w + 1])
                hi = min((w + 1) * P, NB)
                if hi > w * P:
                    nc.sync.dma_start(ag_in[0][w * P:hi, :],
                                      hs_sb[0][:hi - w * P, sl])

            def allgather(l):
                if single_core:
                    nc.sync.dma_start(tbl[l][0:NB, :], ag_in[l][:, :])
                else:
                    nc.gpsimd.collective_compute(
                        "AllGather", mybir.AluOpType.bypass,
                        ins=[ag_in[l][:, :]],
                        outs=[tbl[l][0:N_NODES, :]],
                        replica_groups=[list(range(NCORES))])

            allgather(0)

            # ---- aggregation layers
            ccums_lo = np.concatenate([[0], np.cumsum(CWlo)]).astype(int)
            ccums_hi = np.concatenate([[0], np.cumsum(CWhi)]).astype(int)
            pool_ps = ps_aux.tile([P, DIM], fp32, space="PSUM", tag="aux")

            for l in range(2):
                for (w0, nw, c0l, ncl, c0h, nch) in batches:
                    gt = {}
                    for h, c0, ncols, pool_, tcol in (
                            ("lo", c0l, ncl, glo_pool, TClo),
                            ("hi", c0h, nch, ghi_pool, TChi)):
                        g = pool_.tile([P, ncols, 2 * DIM], fp16, tag="g",
                                       name=f"g{h}_{l}_{w0}")
                        nc.gpsimd.dma_gather(
                            out_ap=g[:, :, :], in_ap=gview[h][l],
                            idxs_ap=idx_sb[h][:, 8 * c0:8 * (c0 + ncols)],
                            num_idxs=P * ncols, num_idxs_reg=P * ncols,
                            elem_size=2 * DIM, elem_step=DIM,
                            single_packet=False)
                        gt[h] = g

                    for w in range(w0, w0 + nw):
                        sl = slice(w * P, (w + 1) * P)
                        po = ps_out.tile([P, DIM], fp32, space="PSUM", tag="po")
                        nc.tensor.matmul(out=po[:],
                                         lhsT=sqd_sb[0:1, sl],
                                         rhs=b_sb[l][:], start=True, stop=False)
                        nc.tensor.matmul(out=po[:], lhsT=id16_sb[:],
                                         rhs=hs_sb[l][:, sl],
                                         start=False, stop=False)
                        for h, c0, ccums in (("lo", c0l, ccums_lo),
                                             ("hi", c0h, ccums_hi)):
                            j0 = int(ccums[w]) - c0
                            cw = int(ccums[w + 1] - ccums[w])
                            last = h == "hi"
                            for j in range(cw):
                                C = cpool.tile([P, P], fp16, tag="C")
                                nc.vector.tensor_scalar(
                                    out=C[:], in0=iota16_sb[:],
                                    scalar1=no_sb[h][:, c0 + j0 + j:
                                                     c0 + j0 + j + 1],
                                    scalar2=None,
                                    op0=mybir.AluOpType.is_equal)
                                nc.tensor.matmul(
                                    out=po[:], lhsT=C[:],
                                    rhs=gt[h][:, j0 + j, 0:DIM],
                                    start=False,
                                    stop=last and (j == cw - 1))
                        # epilogue
                        xn = wk.tile([P, DIM], fp16, tag="xn")
                        nc.scalar.activation(
                            out=xn[:], in_=po[:],
                            func=mybir.ActivationFunctionType.Relu,
                            scale=dinv_sb[:, w:w + 1])
                        if l == 0:
                            ptr = ps_feat.tile([P, DIM], fp16, space="PSUM",
                                               tag="ph")
                            nc.tensor.transpose(out=ptr[:], in_=xn[:],
                                                identity=id16_sb[:])
                            x2t = wk.tile([P, DIM], fp16, tag="x2t")
                            nc.scalar.activation(
                                out=x2t[:], in_=ptr[:],
                                func=mybir.ActivationFunctionType.Copy)
                            ph2 = ps_feat.tile([P, DIM], fp32, space="PSUM",
                                               tag="ph")
                            nc.tensor.matmul(out=ph2[:], lhsT=x2t[:],
                                             rhs=W_sb[1][:],
                                             start=True, stop=True)
                            nc.scalar.activation(
                                out=hs_sb[1][:, sl], in_=ph2[:],
                                func=mybir.ActivationFunctionType.Copy,
                                scale=dinv_sb[:, w:w + 1])
                            hi = min((w + 1) * P, NB)
                            if hi > w * P:
                                nc.sync.dma_start(ag_in[1][w * P:hi, :],
                                                  hs_sb[1][:hi - w * P, sl])
                        else:
                            Cg = wk.tile([P, P], fp16, tag="Cg")
                            nc.vector.tensor_scalar(
                                out=Cg[:], in0=iota16_sb[:],
                                scalar1=xbs_sb[:, w:w + 1],
                                scalar2=None, op0=mybir.AluOpType.is_equal)
                            nc.tensor.matmul(out=pool_ps[:], lhsT=Cg[:],
                                             rhs=xn[:], start=(w == 0),
                                             stop=(w == WPC - 1))
                if l == 0:
                    allgather(1)

            # ---- pooling scatter + AllReduce
            pool_sb = wk.tile([P, DIM], fp32)
            nc.vector.tensor_copy(pool_sb[:], pool_ps[:])
            for b4 in range(4):
                S = wk.tile([P, P], fp32, tag="S")
                nc.vector.tensor_scalar(
                    out=S[:], in0=iota_sb[:],
                    scalar1=pools_sb[:, b4:b4 + 1],
                    scalar2=None, op0=mybir.AluOpType.is_equal)
                pblk = ps_feat.tile([P, DIM], fp32, space="PSUM", tag="ph")
                nc.tensor.matmul(out=pblk[:], lhsT=S[:], rhs=pool_sb[:],
                                 start=True, stop=True)
                sblk = wk.tile([P, DIM], fp32, tag="sblk")
                nc.vector.tensor_copy(sblk[:], pblk[:])
                nc.sync.dma_start(ar_in[b4 * P:(b4 + 1) * P, :], sblk[:])
            if single_core:
                nc.sync.dma_start(ar_out[:, :], ar_in[:, :])
            else:
                nc.gpsimd.collective_compute(
                    "AllReduce", mybir.AluOpType.add,
                    ins=[ar_in[:, :]], outs=[ar_out[:, :]],
                    replica_groups=[list(range(NCORES))])

            # ---- head
            pooledT = bigp.tile([P, NUM_GRAPHS], fp32)
            for b4 in range(4):
                pt = wk.tile([P, DIM], fp32, tag="pt")
                nc.sync.dma_start(pt[:], ar_out[b4 * P:(b4 + 1) * P, :])
                ptp = ps_feat.tile([P, DIM], fp32, space="PSUM", tag="ph")
                nc.tensor.transpose(out=ptp[:], in_=pt[:], identity=id_sb[:])
                nc.vector.tensor_copy(pooledT[:, b4 * P:(b4 + 1) * P], ptp[:])
            lgp = ps_aux.tile([DIM_OUT, NUM_GRAPHS], fp32, space="PSUM",
                              tag="aux")
            nc.tensor.matmul(out=lgp[:], lhsT=Wh_sb[:], rhs=pooledT[:],
                             start=True, stop=False)
            nc.tensor.matmul(out=lgp[:], lhsT=bh_sb[:], rhs=ones_sb[:],
                             start=False, stop=True)
            lgT = wk.tile([DIM_OUT, NUM_GRAPHS], fp32)
            nc.vector.tensor_copy(lgT[:], lgp[:])
            for b4 in range(4):
                lp = ps_feat.tile([P, DIM_OUT], fp32, space="PSUM", tag="ph")
                nc.tensor.transpose(out=lp[:],
                                    in_=lgT[:, b4 * P:(b4 + 1) * P],
                                    identity=id_sb[:DIM_OUT, :DIM_OUT])
                lg = wk.tile([P, DIM_OUT], fp32, tag="lg")
                nc.vector.tensor_copy(lg[:], lp[:])
                # log_softmax over the 64 classes (free dim)
                m = wk.tile([P, 1], fp32, tag="m")
                nc.vector.tensor_reduce(out=m[:], in_=lg[:],
                                        op=mybir.AluOpType.max,
                                        axis=mybir.AxisListType.X)
                mneg = wk.tile([P, 1], fp32, tag="mneg")
                nc.vector.tensor_scalar(out=mneg[:], in0=m[:], scalar1=-1.0,
                                        scalar2=None,
                                        op0=mybir.AluOpType.mult)
                e = wk.tile([P, DIM_OUT], fp32, tag="e")
                se = wk.tile([P, 1], fp32, tag="se")
                nc.scalar.activation(out=e[:], in_=lg[:],
                                     func=mybir.ActivationFunctionType.Exp,
                                     bias=mneg[:, :1], accum_out=se[:])
                lse = wk.tile([P, 1], fp32, tag="lse")
                nc.scalar.activation(out=lse[:], in_=se[:],
                                     func=mybir.ActivationFunctionType.Ln)
                off = wk.tile([P, 1], fp32, tag="off")
                nc.vector.tensor_tensor(out=off[:], in0=lse[:], in1=m[:],
                                        op=mybir.AluOpType.add)
                o = wk.tile([P, DIM_OUT], fp32, tag="o")
                nc.vector.tensor_scalar(out=o[:], in0=lg[:],
                                        scalar1=off[:, :1], scalar2=None,
                                        op0=mybir.AluOpType.subtract)
                nc.sync.dma_start(d_out[b4 * P:(b4 + 1) * P, :], o[:])

    nc.compile()
    return nc


# ---------------------------------------------------------------- entry
def kernel(x, edge_index, x_batch, W1, b1, W2, b2, Wh, bh):
    """Full-input GCN kernel: shards nodes/edges across 8 NeuronCores."""
    from concourse.bass_utils import run_bass_kernel_spmd

    per_core, shared = preprocess(x, edge_index, x_batch)
    consts = const_inputs(W1, b1, W2, b2, Wh, bh)
    in_maps = [{**pc, **consts} for pc in per_core]
    nc = build_kernel(shared)
    declared = set()
    for alloc in nc.m.functions[0].allocations:
        if isinstance(alloc, mybir.MemoryLocationSet) and \
                alloc.kind == "ExternalInput":
            declared.add(alloc.memorylocations[0].name)
    in_maps = [{k: v for k, v in m.items() if k in declared} for m in in_maps]
    res = run_bass_kernel_spmd(nc, in_maps, core_ids=list(range(NCORES)))
    return np.asarray(res.results[0]["out"], dtype=np.float32)


# revision 9
# speedup vs baseline: 4.5751x; 1.0898x over previous
"""GCN (2-layer GCNConv + global_add_pool + dense head) on 8 TRN2 cores.

Strategy (graph/data parallel, per sharding hint):
 - Nodes block-partitioned: core c owns rows [6250c, 6250(c+1)).
 - Edges partitioned by destination block, sorted by (dst window, src half).
 - Per layer: h = x @ W in fp16, hs = h * dinv folded on the Act engine,
   AllGather hs -> full fp16 table in every core's HBM.
 - Aggregation: batched dma_gather (one 512B descriptor per edge) using an
   overlapping-stride table view (elem_step = 1 row, elem = 2 rows) so each
   slot holds its true source row; int16 index range handled by splitting
   the table into lo/hi half views.  Per 128-slot chunk: one one-hot matmul
   (C built on DVE via is_equal) accumulated into the window's PSUM.
 - Self-loop contribution via identity matmul of hs; bias via K=1 matmul of
   sqrt(deg) x b.  Epilogue relu(po * dinv) on Act; layer-1 epilogue also
   transposes x2 (PE) and immediately runs the layer-2 feature matmul.
 - Layer-2 windows accumulate global_add_pool via one-hot graph matmuls;
   pooled partials are scattered to graph rows, AllReduced, and the dense
   head + log_softmax runs redundantly on every core.
"""
import sys

sys.path.insert(0, "/opt/trn_rl_repo")

import math
import numpy as np

import concourse.bacc as bacc
import concourse.bass as bass
import concourse.mybir as mybir
import concourse.tile as tile

P = 128
N_NODES = 50000
N_EDGES = 640000
DIM = 128
DIM_OUT = 64
NUM_GRAPHS = 512
NCORES = 8
NB = N_NODES // NCORES          # 6250 nodes per core
WPC = math.ceil(NB / P)         # 49 windows per core
NBP = WPC * P                   # 6272 padded
HALFP = 25088                   # permuted-subrow split (= 512 * WPC)
TPR = NCORES * P                # 1024 table partition-rows
WGRP = 4                        # windows per gather batch

fp32 = mybir.dt.float32
fp16 = mybir.dt.float16
i16 = mybir.dt.int16


# ---------------------------------------------------------------- host prep
def preprocess(x, edge_index, x_batch):
    src = np.asarray(edge_index[0], dtype=np.int64)
    dst = np.asarray(edge_index[1], dtype=np.int64)
    xb = np.asarray(x_batch, dtype=np.int64)
    x = np.asarray(x, dtype=np.float32)

    edeg = np.bincount(dst, minlength=N_NODES)
    deg = 1.0 + edeg.astype(np.float32)
    dinv = (1.0 / np.sqrt(deg)).astype(np.float32)
    sqd = np.sqrt(deg).astype(np.float32)

    order = np.argsort(dst, kind="stable")
    src_sorted = src[order]
    starts = np.zeros(N_NODES + 1, np.int64)
    np.cumsum(edeg, out=starts[1:])

    # per (core, window, half) edge lists; chunk grid = max over cores
    lists = [[None] * WPC for _ in range(NCORES)]
    cwlo = np.zeros((NCORES, WPC), np.int64)
    cwhi = np.zeros((NCORES, WPC), np.int64)
    for c in range(NCORES):
        b = c * NB
        for w in range(WPC):
            lo, hi = b + w * P, b + min((w + 1) * P, NB)
            srcs_w = src_sorted[starts[lo]:starts[hi]]
            nloc_w = np.repeat(np.arange(hi - lo), edeg[lo:hi])
            # permuted table subrow: node n -> (core, part, window) subrow id
            cc, rr = srcs_w // NB, srcs_w % NB
            pr = (cc * P + rr % P) * WPC + rr // P
            m = pr < HALFP
            lists[c][w] = (pr[m], nloc_w[m], pr[~m] - HALFP, nloc_w[~m])
            cwlo[c, w] = max(1, math.ceil(int(m.sum()) / P))
            cwhi[c, w] = max(1, math.ceil(int((~m).sum()) / P))
    CWlo = cwlo.max(axis=0)
    CWhi = cwhi.max(axis=0)
    TClo = int(CWlo.sum())
    TChi = int(CWhi.sum())

    def wrap16(flat):
        # index i -> [i % 16, i // 16], replicated across 128 partitions
        n = len(flat)
        arr = np.zeros((P, n // 16), np.int16)
        arr[:16] = flat.reshape(n // 16, 16).T
        for r in range(1, 8):
            arr[16 * r:16 * (r + 1)] = arr[:16]
        return arr

    per_core = []
    for c in range(NCORES):
        b = c * NB
        streams = {}
        for half, CW, TC, ilo in (("lo", CWlo, TClo, 0), ("hi", CWhi, TChi, 2)):
            idxf = np.zeros(TC * P, np.int16)
            nof = np.full(TC * P, -1.0, np.float32)
            col = 0
            for w in range(WPC):
                s, n = lists[c][w][ilo], lists[c][w][ilo + 1]
                o = col * P
                idxf[o:o + len(s)] = s.astype(np.int16)
                nof[o:o + len(n)] = n.astype(np.float16)
                col += int(CW[w])
            streams[f"idx_{half}"] = wrap16(idxf)
            streams[f"no_{half}"] = nof.reshape(TC, P).T.copy()

        nid = b + np.arange(NBP)
        ok = np.arange(NBP) < NB
        nidc = np.minimum(nid, N_NODES - 1)
        dinv_c = np.where(ok, dinv[nidc], 0.0).astype(np.float32)
        sqd_c = np.where(ok, sqd[nidc], 0.0).astype(np.float16)
        gmin = int(xb[b])
        xbs_c = np.where(ok, xb[nidc] - gmin, 200.0).astype(np.float32)
        assert int(xb[b + NB - 1]) - gmin + 1 <= P
        pools = np.stack(
            [gmin + np.arange(P, dtype=np.float32) - P * b4 for b4 in range(4)],
            axis=1,
        ).astype(np.float32)

        xT = np.zeros((DIM, NBP), np.float16)
        xT[:, :NB] = x[b:b + NB].T.astype(np.float16)

        per_core.append(dict(
            xT=xT,
            dinv2d=dinv_c.reshape(WPC, P).T.copy(),
            sqdT=sqd_c.reshape(1, NBP),
            xbshift=xbs_c.reshape(WPC, P).T.copy(),
            pools=pools,
            **streams,
        ))

    shared = dict(CWlo=CWlo, CWhi=CWhi, TClo=TClo, TChi=TChi)
    return per_core, shared


def const_inputs(W1, b1, W2, b2, Wh, bh):
    iota = np.tile(np.arange(P, dtype=np.float32)[None, :], (P, 1))
    return dict(
        iota=iota, iota16=iota.astype(np.float16),
        ident16=np.eye(P, dtype=np.float16),
        ident=np.eye(P, dtype=np.float32),
        W1=np.asarray(W1, np.float16), W2=np.asarray(W2, np.float16),
        Wh=np.asarray(Wh, np.float32),
        b1=np.asarray(b1, np.float16).reshape(1, DIM),
        b2=np.asarray(b2, np.float16).reshape(1, DIM),
        bh=np.asarray(bh, np.float32).reshape(1, DIM_OUT),
        ones512=np.ones((1, NUM_GRAPHS), np.float32),
    )


# ---------------------------------------------------------------- kernel
def build_kernel(shared, single_core=False, wgrp=WGRP):
    CWlo, CWhi = shared["CWlo"], shared["CWhi"]
    TClo, TChi = shared["TClo"], shared["TChi"]

    nc = bacc.Bacc("TRN2", target_bir_lowering=False, debug=False,
                   enable_asserts=False,
                   num_devices=1 if single_core else NCORES)

    # inputs
    d_xT = nc.dram_tensor("xT", [DIM, NBP], fp16, kind="ExternalInput")
    d_idx = {h: nc.dram_tensor(f"idx_{h}", [P, tc * 8], i16,
                               kind="ExternalInput")
             for h, tc in (("lo", TClo), ("hi", TChi))}
    d_no = {h: nc.dram_tensor(f"no_{h}", [P, tc], fp32, kind="ExternalInput")
            for h, tc in (("lo", TClo), ("hi", TChi))}
    d_dinv = nc.dram_tensor("dinv2d", [P, WPC], fp32, kind="ExternalInput")
    d_sqd = nc.dram_tensor("sqdT", [1, NBP], fp16, kind="ExternalInput")
    d_xbs = nc.dram_tensor("xbshift", [P, WPC], fp32, kind="ExternalInput")
    d_pools = nc.dram_tensor("pools", [P, 4], fp32, kind="ExternalInput")
    d_iota = nc.dram_tensor("iota", [P, P], fp32, kind="ExternalInput")
    d_iota16 = nc.dram_tensor("iota16", [P, P], fp16, kind="ExternalInput")
    d_id16 = nc.dram_tensor("ident16", [P, P], fp16, kind="ExternalInput")
    d_id = nc.dram_tensor("ident", [P, P], fp32, kind="ExternalInput")
    d_W = [nc.dram_tensor("W1", [DIM, DIM], fp16, kind="ExternalInput"),
           nc.dram_tensor("W2", [DIM, DIM], fp16, kind="ExternalInput")]
    d_b = [nc.dram_tensor("b1", [1, DIM], fp16, kind="ExternalInput"),
           nc.dram_tensor("b2", [1, DIM], fp16, kind="ExternalInput")]
    d_Wh = nc.dram_tensor("Wh", [DIM, DIM_OUT], fp32, kind="ExternalInput")
    d_bh = nc.dram_tensor("bh", [1, DIM_OUT], fp32, kind="ExternalInput")
    d_ones = nc.dram_tensor("ones512", [1, NUM_GRAPHS], fp32,
                            kind="ExternalInput")

    d_out = nc.dram_tensor("out", [NUM_GRAPHS, DIM_OUT], fp32,
                           kind="ExternalOutput")

    # internal DRAM (table in permuted [core*P+p, w*DIM+f] layout)
    tbl = [nc.dram_tensor(f"table{l}", [TPR + 1, NBP], fp16,
                          addr_space="Shared")
           for l in range(2)]
    ag_in = [nc.dram_tensor(f"ag_in{l}", [P, NBP], fp16) for l in range(2)]
    ar_in = nc.dram_tensor("ar_in", [NUM_GRAPHS, DIM], fp32)
    ar_out = nc.dram_tensor("ar_out", [NUM_GRAPHS, DIM], fp32,
                            addr_space="Shared")

    # gather batches: [(w0, nw, col0_lo, cols_lo, col0_hi, cols_hi)]
    batches = []
    clo = chi = 0
    for w0 in range(0, WPC, wgrp):
        nw = min(wgrp, WPC - w0)
        cl = int(CWlo[w0:w0 + nw].sum())
        ch = int(CWhi[w0:w0 + nw].sum())
        batches.append((w0, nw, clo, cl, chi, ch))
        clo += cl
        chi += ch

    with tile.TileContext(nc) as tc:
        with tc.tile_pool(name="const", bufs=1) as cst, \
             tc.tile_pool(name="big", bufs=1) as bigp, \
             tc.tile_pool(name="glo", bufs=2) as glo_pool, \
             tc.tile_pool(name="ghi", bufs=2) as ghi_pool, \
             tc.tile_pool(name="cpool", bufs=12) as cpool, \
             tc.tile_pool(name="work", bufs=4) as wk, \
             tc.tile_pool(name="ps_feat", bufs=3, space="PSUM") as ps_feat, \
             tc.tile_pool(name="ps_out", bufs=4, space="PSUM") as ps_out, \
             tc.tile_pool(name="ps_aux", bufs=1, space="PSUM") as ps_aux:

            # ---- constants / inputs to SBUF
            xT_sb = bigp.tile([DIM, NBP], fp16)
            nc.sync.dma_start(xT_sb[:], d_xT[:, :])
            idx_sb = {}
            no_sb = {}
            for h, tc_ in (("lo", TClo), ("hi", TChi)):
                t = bigp.tile([P, tc_ * 8], i16, name=f"idx{h}_sb")
                nc.sync.dma_start(t[:], d_idx[h][:, :])
                idx_sb[h] = t
                t = bigp.tile([P, tc_], fp32, name=f"no{h}_sb")
                nc.sync.dma_start(t[:], d_no[h][:, :])
                no_sb[h] = t
            dinv_sb = cst.tile([P, WPC], fp32)
            nc.sync.dma_start(dinv_sb[:], d_dinv[:, :])
            sqd_sb = cst.tile([1, NBP], fp16)
            nc.sync.dma_start(sqd_sb[:], d_sqd[:, :])
            xbs_sb = cst.tile([P, WPC], fp32)
            nc.sync.dma_start(xbs_sb[:], d_xbs[:, :])
            pools_sb = cst.tile([P, 4], fp32)
            nc.sync.dma_start(pools_sb[:], d_pools[:, :])
            iota_sb = cst.tile([P, P], fp32)
            nc.sync.dma_start(iota_sb[:], d_iota[:, :])
            iota16_sb = cst.tile([P, P], fp16)
            nc.sync.dma_start(iota16_sb[:], d_iota16[:, :])
            id16_sb = cst.tile([P, P], fp16)
            nc.sync.dma_start(id16_sb[:], d_id16[:, :])
            id_sb = cst.tile([P, P], fp32)
            nc.sync.dma_start(id_sb[:], d_id[:, :])
            W_sb = []
            for l in range(2):
                t = cst.tile([DIM, DIM], fp16, name=f"W{l}_sb")
                nc.sync.dma_start(t[:], d_W[l][:, :])
                W_sb.append(t)
            b_sb = []
            for l in range(2):
                t = cst.tile([1, DIM], fp16, name=f"b{l}_sb")
                nc.sync.dma_start(t[:], d_b[l][:, :])
                b_sb.append(t)
            Wh_sb = cst.tile([DIM, DIM_OUT], fp32)
            nc.sync.dma_start(Wh_sb[:], d_Wh[:, :])
            bh_sb = cst.tile([1, DIM_OUT], fp32)
            nc.sync.dma_start(bh_sb[:], d_bh[:, :])
            ones_sb = cst.tile([1, NUM_GRAPHS], fp32)
            nc.sync.dma_start(ones_sb[:], d_ones[:, :])

            hs_sb = [bigp.tile([P, NBP], fp16, name=f"hs{l}") for l in range(2)]

            # overlapped-stride gather views: elem = 2 rows, step = 1 row
            gview = {}
            for h, base in (("lo", 0), ("hi", NCORES * P // 2)):
                gview[h] = [
                    bass.AP(tbl[l][base:, :].tensor, tbl[l][base:, :].offset,
                            [[DIM, HALFP + 1], [1, 2 * DIM]])
                    for l in range(2)
                ]

            def ag_write(l, w):
                # flush hs windows to ag_in in groups of 7 (big descriptors)
                if (w + 1) % 7 == 0:
                    gsl = slice((w - 6) * P, (w + 1) * P)
                    nc.sync.dma_start(ag_in[l][:, gsl], hs_sb[l][:, gsl])

            # ---- layer-1 features: hs1 = (x @ W1) * dinv
            for w in range(WPC):
                sl = slice(w * P, (w + 1) * P)
                ph = ps_feat.tile([P, DIM], fp32, space="PSUM", tag="ph")
                nc.tensor.matmul(out=ph[:], lhsT=xT_sb[:, sl], rhs=W_sb[0][:],
                                 start=True, stop=True)
                nc.vector.tensor_scalar(out=hs_sb[0][:, sl], in0=ph[:],
                                        scalar1=dinv_sb[:, w:w + 1],
                                        scalar2=None,
                                        op0=mybir.AluOpType.mult)
                ag_write(0, w)

            def allgather(l):
                if single_core:
                    nc.sync.dma_start(tbl[l][0:P, :], ag_in[l][:, :])
                else:
                    nc.gpsimd.collective_compute(
                        "AllGather", mybir.AluOpType.bypass,
                        ins=[ag_in[l][:, :]],
                        outs=[tbl[l][0:TPR, :]],
                        replica_groups=[list(range(NCORES))])

            allgather(0)

            # ---- aggregation layers
            ccums_lo = np.concatenate([[0], np.cumsum(CWlo)]).astype(int)
            ccums_hi = np.concatenate([[0], np.cumsum(CWhi)]).astype(int)
            pool_ps = ps_aux.tile([P, DIM], fp32, space="PSUM", tag="aux")

            for l in range(2):
                for (w0, nw, c0l, ncl, c0h, nch) in batches:
                    gt = {}
                    for h, c0, ncols, pool_, tcol in (
                            ("lo", c0l, ncl, glo_pool, TClo),
                            ("hi", c0h, nch, ghi_pool, TChi)):
                        g = pool_.tile([P, ncols, 2 * DIM], fp16, tag="g",
                                       name=f"g{h}_{l}_{w0}")
                        nc.gpsimd.dma_gather(
                            out_ap=g[:, :, :], in_ap=gview[h][l],
                            idxs_ap=idx_sb[h][:, 8 * c0:8 * (c0 + ncols)],
                            num_idxs=P * ncols, num_idxs_reg=P * ncols,
                            elem_size=2 * DIM, elem_step=DIM,
                            single_packet=False)
                        gt[h] = g

                    for w in range(w0, w0 + nw):
                        sl = slice(w * P, (w + 1) * P)
                        po = ps_out.tile([P, DIM], fp32, space="PSUM", tag="po")
                        nc.tensor.matmul(out=po[:],
                                         lhsT=sqd_sb[0:1, sl],
                                         rhs=b_sb[l][:], start=True, stop=False)
                        nc.tensor.matmul(out=po[:], lhsT=id16_sb[:],
                                         rhs=hs_sb[l][:, sl],
                                         start=False, stop=False)
                        for h, c0, ccums in (("lo", c0l, ccums_lo),
                                             ("hi", c0h, ccums_hi)):
                            j0 = int(ccums[w]) - c0
                            cw = int(ccums[w + 1] - ccums[w])
                            last = h == "hi"
                            for j in range(cw):
                                C = cpool.tile([P, P], fp16, tag="C")
                                nc.vector.tensor_scalar(
                                    out=C[:], in0=iota16_sb[:],
                                    scalar1=no_sb[h][:, c0 + j0 + j:
                                                     c0 + j0 + j + 1],
                                    scalar2=None,
                                    op0=mybir.AluOpType.is_equal)
                                nc.tensor.matmul(
                                    out=po[:], lhsT=C[:],
                                    rhs=gt[h][:, j0 + j, 0:DIM],
                                    start=False,
                                    stop=last and (j == cw - 1))
                        # epilogue
                        xn = wk.tile([P, DIM], fp16, tag="xn")
                        nc.scalar.activation(
                            out=xn[:], in_=po[:],
                            func=mybir.ActivationFunctionType.Relu,
                            scale=dinv_sb[:, w:w + 1])
                        if l == 0:
                            ptr = ps_feat.tile([P, DIM], fp16, space="PSUM",
                                               tag="ph")
                            nc.tensor.transpose(out=ptr[:], in_=xn[:],
                                                identity=id16_sb[:])
                            x2t = wk.tile([P, DIM], fp16, tag="x2t")
                            nc.scalar.activation(
                                out=x2t[:], in_=ptr[:],
                                func=mybir.ActivationFunctionType.Copy)
                            ph2 = ps_feat.tile([P, DIM], fp32, space="PSUM",
                                               tag="ph")
                            nc.tensor.matmul(out=ph2[:], lhsT=x2t[:],
                                             rhs=W_sb[1][:],
                                             start=True, stop=True)
                            nc.scalar.activation(
                                out=hs_sb[1][:, sl], in_=ph2[:],
                                func=mybir.ActivationFunctionType.Copy,
                                scale=dinv_sb[:, w:w + 1])
                            ag_write(1, w)
                        else:
                            Cg = wk.tile([P, P], fp16, tag="Cg")
                            nc.vector.tensor_scalar(
                                out=Cg[:], in0=iota16_sb[:],
                                scalar1=xbs_sb[:, w:w + 1],
                                scalar2=None, op0=mybir.AluOpType.is_equal)
                            nc.tensor.matmul(out=pool_ps[:], lhsT=Cg[:],
                                             rhs=xn[:], start=(w == 0),
                                             stop=(w == WPC - 1))
                if l == 0:
                    allgather(1)

            # ---- pooling scatter + AllReduce
            pool_sb = wk.tile([P, DIM], fp32)
            nc.vector.tensor_copy(pool_sb[:], pool_ps[:])
            sblk = wk.tile([P, 4, DIM], fp32)
            for b4 in range(4):
                S = wk.tile([P, P], fp32, tag="S")
                nc.vector.tensor_scalar(
                    out=S[:], in0=iota_sb[:],
                    scalar1=pools_sb[:, b4:b4 + 1],
                    scalar2=None, op0=mybir.AluOpType.is_equal)
                pblk = ps_feat.tile([P, DIM], fp32, space="PSUM", tag="ph")
                nc.tensor.matmul(out=pblk[:], lhsT=S[:], rhs=pool_sb[:],
                                 start=True, stop=True)
                nc.vector.tensor_copy(sblk[:, b4, :], pblk[:])
            ar_in_v = ar_in[:, :].rearrange("(b p) f -> p b f", p=P)
            nc.sync.dma_start(ar_in_v, sblk[:])
            if single_core:
                nc.sync.dma_start(ar_out[:, :], ar_in[:, :])
            else:
                nc.gpsimd.collective_compute(
                    "AllReduce", mybir.AluOpType.add,
                    ins=[ar_in[:, :]], outs=[ar_out[:, :]],
                    replica_groups=[list(range(NCORES))])

            # ---- head
            arT = wk.tile([P, 4, DIM], fp32)
            nc.sync.dma_start(arT[:],
                              ar_out[:, :].rearrange("(b p) f -> p b f", p=P))
            pooledT = bigp.tile([P, NUM_GRAPHS], fp32)
            for b4 in range(4):
                ptp = ps_feat.tile([P, DIM], fp32, space="PSUM", tag="ph")
                nc.tensor.transpose(out=ptp[:], in_=arT[:, b4, :],
                                    identity=id_sb[:])
                nc.vector.tensor_copy(pooledT[:, b4 * P:(b4 + 1) * P], ptp[:])
            lgp = ps_aux.tile([DIM_OUT, NUM_GRAPHS], fp32, space="PSUM",
                              tag="aux")
            nc.tensor.matmul(out=lgp[:], lhsT=Wh_sb[:], rhs=pooledT[:],
                             start=True, stop=False)
            nc.tensor.matmul(out=lgp[:], lhsT=bh_sb[:], rhs=ones_sb[:],
                             start=False, stop=True)
            lgT = wk.tile([DIM_OUT, NUM_GRAPHS], fp32)
            nc.vector.tensor_copy(lgT[:], lgp[:])
            # log_softmax over the 64 classes (free dim); ops grouped by
            # engine + activation function to avoid act-table reloads
            lg4 = wk.tile([P, 4, DIM_OUT], fp32)
            m4 = wk.tile([P, 4], fp32)
            mneg4 = wk.tile([P, 4], fp32)
            e4 = wk.tile([P, 4, DIM_OUT], fp32)
            se4 = wk.tile([P, 4], fp32)
            lse4 = wk.tile([P, 4], fp32)
            off4 = wk.tile([P, 4], fp32)
            o4 = wk.tile([P, 4, DIM_OUT], fp32)
            for b4 in range(4):
                lp = ps_feat.tile([P, DIM_OUT], fp32, space="PSUM", tag="ph")
                nc.tensor.transpose(out=lp[:],
                                    in_=lgT[:, b4 * P:(b4 + 1) * P],
                                    identity=id_sb[:DIM_OUT, :DIM_OUT])
                nc.vector.tensor_copy(lg4[:, b4, :], lp[:])
                nc.vector.tensor_reduce(out=m4[:, b4:b4 + 1],
                                        in_=lg4[:, b4, :],
                                        op=mybir.AluOpType.max,
                                        axis=mybir.AxisListType.X)
            nc.vector.tensor_scalar(out=mneg4[:], in0=m4[:], scalar1=-1.0,
                                    scalar2=None, op0=mybir.AluOpType.mult)
            for b4 in range(4):
                nc.scalar.activation(out=e4[:, b4, :], in_=lg4[:, b4, :],
                                     func=mybir.ActivationFunctionType.Exp,
                                     bias=mneg4[:, b4:b4 + 1],
                                     accum_out=se4[:, b4:b4 + 1])
            nc.scalar.activation(out=lse4[:], in_=se4[:],
                                 func=mybir.ActivationFunctionType.Ln)
            nc.vector.tensor_tensor(out=off4[:], in0=lse4[:], in1=m4[:],
                                    op=mybir.AluOpType.add)
            for b4 in range(4):
                nc.vector.tensor_scalar(out=o4[:, b4, :], in0=lg4[:, b4, :],
                                        scalar1=off4[:, b4:b4 + 1],
                                        scalar2=None,
                                        op0=mybir.AluOpType.subtract)
            nc.sync.dma_start(d_out[:, :].rearrange("(b p) f -> p b f", p=P),
                              o4[:])

    nc.compile()
    return nc


# ---------------------------------------------------------------- entry
def kernel(x, edge_index, x_batch, W1, b1, W2, b2, Wh, bh):
    """Full-input GCN kernel: shards nodes/edges across 8 NeuronCores."""
    from concourse.bass_utils import run_bass_kernel_spmd

    per_core, shared = preprocess(x, edge_index, x_batch)
    consts = const_inputs(W1, b1, W2, b2, Wh, bh)
    in_maps = [{**pc, **consts} for pc in per_core]
    nc = build_kernel(shared)
    declared = set()
    for alloc in nc.m.functions[0].allocations:
        if isinstance(alloc, mybir.MemoryLocationSet) and \
                alloc.kind == "ExternalInput":
            declared.add(alloc.memorylocations[0].name)
    in_maps = [{k: v for k, v in m.items() if k in declared} for m in in_maps]
    res = run_bass_kernel_spmd(nc, in_maps, core_ids=list(range(NCORES)))
    return np.asarray(res.results[0]["out"], dtype=np.float32)


# revision 10
# speedup vs baseline: 4.8892x; 1.0686x over previous
"""GCN (2-layer GCNConv + global_add_pool + dense head) on 8 TRN2 cores.

Strategy (graph/data parallel, per sharding hint):
 - Nodes block-partitioned: core c owns rows [6250c, 6250(c+1)).
 - Edges partitioned by destination block, sorted by (dst window, src half).
 - Per layer: h = x @ W in fp16, hs = h * dinv folded on the Act engine,
   AllGather hs -> full fp16 table in every core's HBM.
 - Aggregation: batched dma_gather (one 512B descriptor per edge) using an
   overlapping-stride table view (elem_step = 1 row, elem = 2 rows) so each
   slot holds its true source row; int16 index range handled by splitting
   the table into lo/hi half views.  Per 128-slot chunk: one one-hot matmul
   (C built on DVE via is_equal) accumulated into the window's PSUM.
 - Self-loop contribution via identity matmul of hs; bias via K=1 matmul of
   sqrt(deg) x b.  Epilogue relu(po * dinv) on Act; layer-1 epilogue also
   transposes x2 (PE) and immediately runs the layer-2 feature matmul.
 - Layer-2 windows accumulate global_add_pool via one-hot graph matmuls;
   pooled partials are scattered to graph rows, AllReduced, and the dense
   head + log_softmax runs redundantly on every core.
"""
import sys

sys.path.insert(0, "/opt/trn_rl_repo")

import math
import numpy as np

import concourse.bacc as bacc
import concourse.bass as bass
import concourse.mybir as mybir
import concourse.tile as tile

P = 128
N_NODES = 50000
N_EDGES = 640000
DIM = 128
DIM_OUT = 64
NUM_GRAPHS = 512
NCORES = 8
NB = N_NODES // NCORES          # 6250 nodes per core
WPC = math.ceil(NB / P)         # 49 windows per core
NBP = WPC * P                   # 6272 padded
HALFP = 25088                   # permuted-subrow split (= 512 * WPC)
TPR = NCORES * P                # 1024 table partition-rows
WGRP = 4                        # windows per gather batch

fp32 = mybir.dt.float32
fp16 = mybir.dt.float16
i16 = mybir.dt.int16


# ---------------------------------------------------------------- host prep
def preprocess(x, edge_index, x_batch):
    src = np.asarray(edge_index[0], dtype=np.int64)
    dst = np.asarray(edge_index[1], dtype=np.int64)
    xb = np.asarray(x_batch, dtype=np.int64)
    x = np.asarray(x, dtype=np.float32)

    edeg = np.bincount(dst, minlength=N_NODES)
    deg = 1.0 + edeg.astype(np.float32)
    dinv = (1.0 / np.sqrt(deg)).astype(np.float32)
    sqd = np.sqrt(deg).astype(np.float32)

    order = np.argsort(dst, kind="stable")
    src_sorted = src[order]
    starts = np.zeros(N_NODES + 1, np.int64)
    np.cumsum(edeg, out=starts[1:])

    # per (core, window, half) edge lists; chunk grid = max over cores
    lists = [[None] * WPC for _ in range(NCORES)]
    cwlo = np.zeros((NCORES, WPC), np.int64)
    cwhi = np.zeros((NCORES, WPC), np.int64)
    for c in range(NCORES):
        b = c * NB
        for w in range(WPC):
            lo, hi = b + w * P, b + min((w + 1) * P, NB)
            srcs_w = src_sorted[starts[lo]:starts[hi]]
            nloc_w = np.repeat(np.arange(hi - lo), edeg[lo:hi])
            # permuted table subrow: node n -> (core, part, window) subrow id
            cc, rr = srcs_w // NB, srcs_w % NB
            pr = (cc * P + rr % P) * WPC + rr // P
            m = pr < HALFP
            lists[c][w] = (pr[m], nloc_w[m], pr[~m] - HALFP, nloc_w[~m])
            cwlo[c, w] = max(1, math.ceil(int(m.sum()) / P))
            cwhi[c, w] = max(1, math.ceil(int((~m).sum()) / P))
    CWlo = cwlo.max(axis=0)
    CWhi = cwhi.max(axis=0)
    TClo = int(CWlo.sum())
    TChi = int(CWhi.sum())

    def wrap16(flat):
        # index i -> [i % 16, i // 16], replicated across 128 partitions
        n = len(flat)
        arr = np.zeros((P, n // 16), np.int16)
        arr[:16] = flat.reshape(n // 16, 16).T
        for r in range(1, 8):
            arr[16 * r:16 * (r + 1)] = arr[:16]
        return arr

    per_core = []
    for c in range(NCORES):
        b = c * NB
        streams = {}
        for half, CW, TC, ilo in (("lo", CWlo, TClo, 0), ("hi", CWhi, TChi, 2)):
            idxf = np.zeros(TC * P, np.int16)
            nof = np.full(TC * P, -1.0, np.float32)
            col = 0
            for w in range(WPC):
                s, n = lists[c][w][ilo], lists[c][w][ilo + 1]
                o = col * P
                idxf[o:o + len(s)] = s.astype(np.int16)
                nof[o:o + len(n)] = n.astype(np.float16)
                col += int(CW[w])
            streams[f"idx_{half}"] = wrap16(idxf)
            streams[f"no_{half}"] = nof.reshape(TC, P).T.copy()

        nid = b + np.arange(NBP)
        ok = np.arange(NBP) < NB
        nidc = np.minimum(nid, N_NODES - 1)
        dinv_c = np.where(ok, dinv[nidc], 0.0).astype(np.float32)
        sqd_c = np.where(ok, sqd[nidc], 0.0).astype(np.float16)
        gmin = int(xb[b])
        xbs_c = np.where(ok, xb[nidc] - gmin, 200.0).astype(np.float32)
        assert int(xb[b + NB - 1]) - gmin + 1 <= P
        pools = np.stack(
            [gmin + np.arange(P, dtype=np.float32) - P * b4 for b4 in range(4)],
            axis=1,
        ).astype(np.float32)

        xT = np.zeros((DIM, NBP), np.float16)
        xT[:, :NB] = x[b:b + NB].T.astype(np.float16)

        per_core.append(dict(
            xT=xT,
            dinv2d=dinv_c.reshape(WPC, P).T.copy(),
            sqdT=sqd_c.reshape(1, NBP),
            xbshift=xbs_c.reshape(WPC, P).T.copy(),
            pools=pools,
            **streams,
        ))

    shared = dict(CWlo=CWlo, CWhi=CWhi, TClo=TClo, TChi=TChi)
    return per_core, shared


def const_inputs(W1, b1, W2, b2, Wh, bh):
    iota = np.tile(np.arange(P, dtype=np.float32)[None, :], (P, 1))
    return dict(
        iota=iota, iota16=iota.astype(np.float16),
        ident16=np.eye(P, dtype=np.float16),
        ident=np.eye(P, dtype=np.float32),
        W1=np.asarray(W1, np.float16), W2=np.asarray(W2, np.float16),
        Wh=np.asarray(Wh, np.float32),
        b1=np.asarray(b1, np.float16).reshape(1, DIM),
        b2=np.asarray(b2, np.float16).reshape(1, DIM),
        bh=np.asarray(bh, np.float32).reshape(1, DIM_OUT),
        ones512=np.ones((1, NUM_GRAPHS), np.float32),
    )


# ---------------------------------------------------------------- kernel
def build_kernel(shared, single_core=False, wgrp=WGRP):
    CWlo, CWhi = shared["CWlo"], shared["CWhi"]
    TClo, TChi = shared["TClo"], shared["TChi"]

    nc = bacc.Bacc("TRN2", target_bir_lowering=False, debug=False,
                   enable_asserts=False,
                   num_devices=1 if single_core else NCORES)

    # inputs
    d_xT = nc.dram_tensor("xT", [DIM, NBP], fp16, kind="ExternalInput")
    d_idx = {h: nc.dram_tensor(f"idx_{h}", [P, tc * 8], i16,
                               kind="ExternalInput")
             for h, tc in (("lo", TClo), ("hi", TChi))}
    d_no = {h: nc.dram_tensor(f"no_{h}", [P, tc], fp32, kind="ExternalInput")
            for h, tc in (("lo", TClo), ("hi", TChi))}
    d_dinv = nc.dram_tensor("dinv2d", [P, WPC], fp32, kind="ExternalInput")
    d_sqd = nc.dram_tensor("sqdT", [1, NBP], fp16, kind="ExternalInput")
    d_xbs = nc.dram_tensor("xbshift", [P, WPC], fp32, kind="ExternalInput")
    d_pools = nc.dram_tensor("pools", [P, 4], fp32, kind="ExternalInput")
    d_iota = nc.dram_tensor("iota", [P, P], fp32, kind="ExternalInput")
    d_iota16 = nc.dram_tensor("iota16", [P, P], fp16, kind="ExternalInput")
    d_id16 = nc.dram_tensor("ident16", [P, P], fp16, kind="ExternalInput")
    d_id = nc.dram_tensor("ident", [P, P], fp32, kind="ExternalInput")
    d_W = [nc.dram_tensor("W1", [DIM, DIM], fp16, kind="ExternalInput"),
           nc.dram_tensor("W2", [DIM, DIM], fp16, kind="ExternalInput")]
    d_b = [nc.dram_tensor("b1", [1, DIM], fp16, kind="ExternalInput"),
           nc.dram_tensor("b2", [1, DIM], fp16, kind="ExternalInput")]
    d_Wh = nc.dram_tensor("Wh", [DIM, DIM_OUT], fp32, kind="ExternalInput")
    d_bh = nc.dram_tensor("bh", [1, DIM_OUT], fp32, kind="ExternalInput")
    d_ones = nc.dram_tensor("ones512", [1, NUM_GRAPHS], fp32,
                            kind="ExternalInput")

    d_out = nc.dram_tensor("out", [NUM_GRAPHS, DIM_OUT], fp32,
                           kind="ExternalOutput")

    # internal DRAM (table in permuted [core*P+p, w*DIM+f] layout)
    tbl = [nc.dram_tensor(f"table{l}", [TPR + 1, NBP], fp16,
                          addr_space="Shared")
           for l in range(2)]
    ag_in = [nc.dram_tensor(f"ag_in{l}", [P, NBP], fp16) for l in range(2)]
    ar_in = nc.dram_tensor("ar_in", [NUM_GRAPHS, DIM], fp32)
    ar_out = nc.dram_tensor("ar_out", [NUM_GRAPHS, DIM], fp32,
                            addr_space="Shared")

    # gather batches: [(w0, nw, col0_lo, cols_lo, col0_hi, cols_hi)]
    batches = []
    clo = chi = 0
    for w0 in range(0, WPC, wgrp):
        nw = min(wgrp, WPC - w0)
        cl = int(CWlo[w0:w0 + nw].sum())
        ch = int(CWhi[w0:w0 + nw].sum())
        batches.append((w0, nw, clo, cl, chi, ch))
        clo += cl
        chi += ch

    with tile.TileContext(nc) as tc:
        with tc.tile_pool(name="const", bufs=1) as cst, \
             tc.tile_pool(name="big", bufs=1) as bigp, \
             tc.tile_pool(name="glo", bufs=3) as glo_pool, \
             tc.tile_pool(name="ghi", bufs=3) as ghi_pool, \
             tc.tile_pool(name="cpool", bufs=12) as cpool, \
             tc.tile_pool(name="work", bufs=4) as wk, \
             tc.tile_pool(name="ps_feat", bufs=3, space="PSUM") as ps_feat, \
             tc.tile_pool(name="ps_out", bufs=4, space="PSUM") as ps_out, \
             tc.tile_pool(name="ps_aux", bufs=1, space="PSUM") as ps_aux:

            # ---- constants / inputs to SBUF
            xT_sb = bigp.tile([DIM, NBP], fp16)
            nc.sync.dma_start(xT_sb[:], d_xT[:, :])
            idx_sb = {}
            no_sb = {}
            for h, tc_ in (("lo", TClo), ("hi", TChi)):
                t = bigp.tile([P, tc_ * 8], i16, name=f"idx{h}_sb")
                nc.sync.dma_start(t[:], d_idx[h][:, :])
                idx_sb[h] = t
                t = bigp.tile([P, tc_], fp32, name=f"no{h}_sb")
                nc.sync.dma_start(t[:], d_no[h][:, :])
                no_sb[h] = t
            dinv_sb = cst.tile([P, WPC], fp32)
            nc.sync.dma_start(dinv_sb[:], d_dinv[:, :])
            sqd_sb = cst.tile([1, NBP], fp16)
            nc.sync.dma_start(sqd_sb[:], d_sqd[:, :])
            xbs_sb = cst.tile([P, WPC], fp32)
            nc.sync.dma_start(xbs_sb[:], d_xbs[:, :])
            pools_sb = cst.tile([P, 4], fp32)
            nc.sync.dma_start(pools_sb[:], d_pools[:, :])
            iota_sb = cst.tile([P, P], fp32)
            nc.sync.dma_start(iota_sb[:], d_iota[:, :])
            iota16_sb = cst.tile([P, P], fp16)
            nc.sync.dma_start(iota16_sb[:], d_iota16[:, :])
            id16_sb = cst.tile([P, P], fp16)
            nc.sync.dma_start(id16_sb[:], d_id16[:, :])
            id_sb = cst.tile([P, P], fp32)
            nc.sync.dma_start(id_sb[:], d_id[:, :])
            W_sb = []
            for l in range(2):
                t = cst.tile([DIM, DIM], fp16, name=f"W{l}_sb")
                nc.sync.dma_start(t[:], d_W[l][:, :])
                W_sb.append(t)
            b_sb = []
            for l in range(2):
                t = cst.tile([1, DIM], fp16, name=f"b{l}_sb")
                nc.sync.dma_start(t[:], d_b[l][:, :])
                b_sb.append(t)
            Wh_sb = cst.tile([DIM, DIM_OUT], fp32)
            nc.sync.dma_start(Wh_sb[:], d_Wh[:, :])
            bh_sb = cst.tile([1, DIM_OUT], fp32)
            nc.sync.dma_start(bh_sb[:], d_bh[:, :])
            ones_sb = cst.tile([1, NUM_GRAPHS], fp32)
            nc.sync.dma_start(ones_sb[:], d_ones[:, :])

            hs_sb = [bigp.tile([P, NBP], fp16, name=f"hs{l}") for l in range(2)]

            # overlapped-stride gather views: elem = 2 rows, step = 1 row
            gview = {}
            for h, base in (("lo", 0), ("hi", NCORES * P // 2)):
                gview[h] = [
                    bass.AP(tbl[l][base:, :].tensor, tbl[l][base:, :].offset,
                            [[DIM, HALFP + 1], [1, 2 * DIM]])
                    for l in range(2)
                ]

            def ag_write(l, w):
                # flush hs windows to ag_in in groups of 7 (big descriptors)
                if (w + 1) % 7 == 0:
                    gsl = slice((w - 6) * P, (w + 1) * P)
                    nc.sync.dma_start(ag_in[l][:, gsl], hs_sb[l][:, gsl])

            # ---- layer-1 features: hs1 = (x @ W1) * dinv
            for w in range(WPC):
                sl = slice(w * P, (w + 1) * P)
                ph = ps_feat.tile([P, DIM], fp32, space="PSUM", tag="ph")
                nc.tensor.matmul(out=ph[:], lhsT=xT_sb[:, sl], rhs=W_sb[0][:],
                                 start=True, stop=True)
                nc.vector.tensor_scalar(out=hs_sb[0][:, sl], in0=ph[:],
                                        scalar1=dinv_sb[:, w:w + 1],
                                        scalar2=None,
                                        op0=mybir.AluOpType.mult)
                ag_write(0, w)

            def allgather(l):
                if single_core:
                    nc.sync.dma_start(tbl[l][0:P, :], ag_in[l][:, :])
                else:
                    nc.gpsimd.collective_compute(
                        "AllGather", mybir.AluOpType.bypass,
                        ins=[ag_in[l][:, :]],
                        outs=[tbl[l][0:TPR, :]],
                        replica_groups=[list(range(NCORES))])

            allgather(0)

            # ---- aggregation layers
            ccums_lo = np.concatenate([[0], np.cumsum(CWlo)]).astype(int)
            ccums_hi = np.concatenate([[0], np.cumsum(CWhi)]).astype(int)
            pool_ps = ps_aux.tile([P, DIM], fp32, space="PSUM", tag="aux")

            for l in range(2):
                for (w0, nw, c0l, ncl, c0h, nch) in batches:
                    gt = {}
                    for h, c0, ncols, pool_, tcol in (
                            ("lo", c0l, ncl, glo_pool, TClo),
                            ("hi", c0h, nch, ghi_pool, TChi)):
                        g = pool_.tile([P, ncols, 2 * DIM], fp16, tag="g",
                                       name=f"g{h}_{l}_{w0}")
                        nc.gpsimd.dma_gather(
                            out_ap=g[:, :, :], in_ap=gview[h][l],
                            idxs_ap=idx_sb[h][:, 8 * c0:8 * (c0 + ncols)],
                            num_idxs=P * ncols, num_idxs_reg=P * ncols,
                            elem_size=2 * DIM, elem_step=DIM,
                            single_packet=False)
                        gt[h] = g

                    for w in range(w0, w0 + nw):
                        sl = slice(w * P, (w + 1) * P)
                        po = ps_out.tile([P, DIM], fp32, space="PSUM", tag="po")
                        nc.tensor.matmul(out=po[:],
                                         lhsT=sqd_sb[0:1, sl],
                                         rhs=b_sb[l][:], start=True, stop=False)
                        nc.tensor.matmul(out=po[:], lhsT=id16_sb[:],
                                         rhs=hs_sb[l][:, sl],
                                         start=False, stop=False)
                        for h, c0, ccums in (("lo", c0l, ccums_lo),
                                             ("hi", c0h, ccums_hi)):
                            j0 = int(ccums[w]) - c0
                            cw = int(ccums[w + 1] - ccums[w])
                            last = h == "hi"
                            for j in range(cw):
                                C = cpool.tile([P, P], fp16, tag="C")
                                nc.vector.tensor_scalar(
                                    out=C[:], in0=iota16_sb[:],
                                    scalar1=no_sb[h][:, c0 + j0 + j:
                                                     c0 + j0 + j + 1],
                                    scalar2=None,
                                    op0=mybir.AluOpType.is_equal)
                                nc.tensor.matmul(
                                    out=po[:], lhsT=C[:],
                                    rhs=gt[h][:, j0 + j, 0:DIM],
                                    start=False,
                                    stop=last and (j == cw - 1))
                        # epilogue
                        xn = wk.tile([P, DIM], fp16, tag="xn")
                        nc.scalar.activation(
                            out=xn[:], in_=po[:],
                            func=mybir.ActivationFunctionType.Relu,
                            scale=dinv_sb[:, w:w + 1])
                        if l == 0:
                            ptr = ps_feat.tile([P, DIM], fp16, space="PSUM",
                                               tag="ph")
                            nc.tensor.transpose(out=ptr[:], in_=xn[:],
                                                identity=id16_sb[:])
                            x2t = wk.tile([P, DIM], fp16, tag="x2t")
                            nc.scalar.activation(
                                out=x2t[:], in_=ptr[:],
                                func=mybir.ActivationFunctionType.Copy)
                            ph2 = ps_feat.tile([P, DIM], fp32, space="PSUM",
                                               tag="ph")
                            nc.tensor.matmul(out=ph2[:], lhsT=x2t[:],
                                             rhs=W_sb[1][:],
                                             start=True, stop=True)
                            nc.scalar.activation(
                                out=hs_sb[1][:, sl], in_=ph2[:],
                                func=mybir.ActivationFunctionType.Copy,
                                scale=dinv_sb[:, w:w + 1])
                            ag_write(1, w)
                        else:
                            Cg = wk.tile([P, P], fp16, tag="Cg")
                            nc.vector.tensor_scalar(
                                out=Cg[:], in0=iota16_sb[:],
                                scalar1=xbs_sb[:, w:w + 1],
                                scalar2=None, op0=mybir.AluOpType.is_equal)
                            nc.tensor.matmul(out=pool_ps[:], lhsT=Cg[:],
                                             rhs=xn[:], start=(w == 0),
                                             stop=(w == WPC - 1))
                if l == 0:
                    allgather(1)

            # ---- pooling scatter + AllReduce
            pool_sb = wk.tile([P, DIM], fp32)
            nc.vector.tensor_copy(pool_sb[:], pool_ps[:])
            sblk = wk.tile([P, 4, DIM], fp32)
            for b4 in range(4):
                S = wk.tile([P, P], fp32, tag="S")
                nc.vector.tensor_scalar(
                    out=S[:], in0=iota_sb[:],
                    scalar1=pools_sb[:, b4:b4 + 1],
                    scalar2=None, op0=mybir.AluOpType.is_equal)
                pblk = ps_feat.tile([P, DIM], fp32, space="PSUM", tag="ph")
                nc.tensor.matmul(out=pblk[:], lhsT=S[:], rhs=pool_sb[:],
                                 start=True, stop=True)
                nc.vector.tensor_copy(sblk[:, b4, :], pblk[:])
            ar_in_v = ar_in[:, :].rearrange("(b p) f -> p b f", p=P)
            nc.sync.dma_start(ar_in_v, sblk[:])
            if single_core:
                nc.sync.dma_start(ar_out[:, :], ar_in[:, :])
            else:
                nc.gpsimd.collective_compute(
                    "AllReduce", mybir.AluOpType.add,
                    ins=[ar_in[:, :]], outs=[ar_out[:, :]],
                    replica_groups=[list(range(NCORES))])

            # ---- head
            arT = wk.tile([P, 4, DIM], fp32)
            nc.sync.dma_start(arT[:],
                              ar_out[:, :].rearrange("(b p) f -> p b f", p=P))
            pooledT = bigp.tile([P, NUM_GRAPHS], fp32)
            for b4 in range(4):
                ptp = ps_feat.tile([P, DIM], fp32, space="PSUM", tag="ph")
                nc.tensor.transpose(out=ptp[:], in_=arT[:, b4, :],
                                    identity=id_sb[:])
                nc.vector.tensor_copy(pooledT[:, b4 * P:(b4 + 1) * P], ptp[:])
            # logits per graph-block directly: [128 g x 64 c] matmuls
            lg4 = wk.tile([P, 4, DIM_OUT], fp32)
            m4 = wk.tile([P, 4], fp32)
            mneg4 = wk.tile([P, 4], fp32)
            e4 = wk.tile([P, 4, DIM_OUT], fp32)
            se4 = wk.tile([P, 4], fp32)
            lse4 = wk.tile([P, 4], fp32)
            off4 = wk.tile([P, 4], fp32)
            o4 = wk.tile([P, 4, DIM_OUT], fp32)
            for b4 in range(4):
                lp = ps_feat.tile([P, DIM_OUT], fp32, space="PSUM", tag="ph")
                nc.tensor.matmul(out=lp[:],
                                 lhsT=pooledT[:, b4 * P:(b4 + 1) * P],
                                 rhs=Wh_sb[:], start=True, stop=False)
                nc.tensor.matmul(out=lp[:], lhsT=ones_sb[0:1, 0:P],
                                 rhs=bh_sb[:], start=False, stop=True)
                nc.vector.tensor_copy(lg4[:, b4, :], lp[:])
                nc.vector.tensor_reduce(out=m4[:, b4:b4 + 1],
                                        in_=lg4[:, b4, :],
                                        op=mybir.AluOpType.max,
                                        axis=mybir.AxisListType.X)
            nc.vector.tensor_scalar(out=mneg4[:], in0=m4[:], scalar1=-1.0,
                                    scalar2=None, op0=mybir.AluOpType.mult)
            for b4 in range(4):
                nc.scalar.activation(out=e4[:, b4, :], in_=lg4[:, b4, :],
                                     func=mybir.ActivationFunctionType.Exp,
                                     bias=mneg4[:, b4:b4 + 1],
                                     accum_out=se4[:, b4:b4 + 1])
            nc.scalar.activation(out=lse4[:], in_=se4[:],
                                 func=mybir.ActivationFunctionType.Ln)
            nc.vector.tensor_tensor(out=off4[:], in0=lse4[:], in1=m4[:],
                                    op=mybir.AluOpType.add)
            for b4 in range(4):
                nc.vector.tensor_scalar(out=o4[:, b4, :], in0=lg4[:, b4, :],
                                        scalar1=off4[:, b4:b4 + 1],
                                        scalar2=None,
                                        op0=mybir.AluOpType.subtract)
            nc.sync.dma_start(d_out[:, :].rearrange("(b p) f -> p b f", p=P),
                              o4[:])

    nc.compile()
    return nc


# ---------------------------------------------------------------- entry
def kernel(x, edge_index, x_batch, W1, b1, W2, b2, Wh, bh):
    """Full-input GCN kernel: shards nodes/edges across 8 NeuronCores."""
    from concourse.bass_utils import run_bass_kernel_spmd

    per_core, shared = preprocess(x, edge_index, x_batch)
    consts = const_inputs(W1, b1, W2, b2, Wh, bh)
    in_maps = [{**pc, **consts} for pc in per_core]
    nc = build_kernel(shared)
    declared = set()
    for alloc in nc.m.functions[0].allocations:
        if isinstance(alloc, mybir.MemoryLocationSet) and \
                alloc.kind == "ExternalInput":
            declared.add(alloc.memorylocations[0].name)
    in_maps = [{k: v for k, v in m.items() if k in declared} for m in in_maps]
    res = run_bass_kernel_spmd(nc, in_maps, core_ids=list(range(NCORES)))
    return np.asarray(res.results[0]["out"], dtype=np.float32)
